# revision 14
# baseline (speedup 1.0000x reference)
"""Bipartite GNN message passing on 8 Trainium2 NeuronCores.

Math reformulation: relu(h[idx] @ W + b) == relu(h @ W + b)[idx], so each
direction-layer is: per-node message MLP (z) -> gather z rows by edge ->
segment-sum -> update MLP.  Sharding: aggregation-side nodes are split into
8 contiguous ranges (one per core); each core owns ALL edges targeting its
range, so it computes complete aggregates locally (no AllReduce).  Only the
small per-shard z tensors are AllGathered (fp16) each direction-layer.

Segment-sum on the tensor engine: edges sorted by (gather-table, subwindow);
for each 128-edge chunk a one-hot S[e,j] = (dst_local[e] == j) matrix (built
with one DVE is_equal against a constant iota tile) turns the segment sum
into  psum[64f, 64dst] += gathered_z[128e, 64f].T @ S[128e, 64dst],
accumulated per 512-dst PSUM window and flushed additively to SBUF.

Gathers use dma_gather (one SWDGE instruction per ~8k rows, int16 indices
into <=25088-row table slices of the AllGathered z, 256B padded fp16 rows).

SPMD: one NEFF for all 8 cores, so the chunk schedule (chunks per
(table, subwindow)) is the max over cores; cores pad with (idx=0,
dst_local=sentinel) edges that contribute zero.
"""
import numpy as np

D = 64
CHUNK = 128
SUB = 64
WIN = 512
SENT = 999.0
N_CORES = 8
GMAX = 1024


class Cfg:
    def __init__(self, ns_pad, nd_pad, n_tab_s, n_tab_d):
        self.NS_PAD, self.ND_PAD = ns_pad, nd_pad
        self.SRC_SH, self.DST_SH = ns_pad // N_CORES, nd_pad // N_CORES
        self.N_TAB_S, self.N_TAB_D = n_tab_s, n_tab_d
        self.TAB_S = ns_pad // n_tab_s
        self.TAB_D = nd_pad // n_tab_d
        assert self.TAB_S <= 32767 and self.TAB_D <= 32767
        assert self.SRC_SH % CHUNK == 0 and self.DST_SH % CHUNK == 0


REAL_CFG = Cfg(100352, 50176, 4, 2)


def _build_plan(gather_idx, seg_idx, table_rows, n_tables, shard):
    """SPMD-uniform edge plan for one direction.

    Returns dict with per-core idx16 [128, TOT/16] int16, dw [128, TOT/128]
    fp16, plus uniform visits [(q, w, [k_per_sub])] and gathers
    [(q, slot0, nslots)].
    """
    n_sub_tot = shard // SUB
    core_of = seg_idx // shard
    per_core = []
    Kmax = np.zeros((n_tables, n_sub_tot), np.int64)
    for c in range(N_CORES):
        m = core_of == c
        g = gather_idx[m]
        s = seg_idx[m] - c * shard
        q = (g // table_rows).astype(np.int64)
        subg = s // SUB
        key = q * n_sub_tot + subg
        order = np.argsort(key, kind="stable")
        g, s, q, subg = g[order], s[order], q[order], subg[order]
        per_core.append((g % table_rows, s, q, subg))
        cnt = np.zeros((n_tables, n_sub_tot), np.int64)
        np.add.at(cnt, (q, subg), 1)
        Kmax = np.maximum(Kmax, (cnt + CHUNK - 1) // CHUNK)

    slots_per_group = Kmax * CHUNK
    flat = slots_per_group.reshape(-1)
    starts = np.concatenate([[0], np.cumsum(flat)[:-1]]).reshape(n_tables, n_sub_tot)
    total = int(flat.sum())
    assert total % CHUNK == 0

    idx16_list, dw_list = [], []
    for c in range(N_CORES):
        g, s, q, subg = per_core[c]
        G = np.zeros(total, np.int32)
        DW = np.full(total, SENT, np.float32)
        kk = q * n_sub_tot + subg
        bounds = np.flatnonzero(np.diff(kk)) + 1
        for grp in np.split(np.arange(len(g)), bounds):
            if len(grp) == 0:
                continue
            qq, sg = int(q[grp[0]]), int(subg[grp[0]])
            st = int(starts[qq, sg])
            n = len(grp)
            G[st:st + n] = g[grp]
            DW[st:st + n] = s[grp] - sg * SUB
        # idx16: j -> [16k + j%16, j//16] replicated for 8 gpsimd cores
        i16 = np.empty((128, total // 16), np.int16)
        base = G.astype(np.int16).reshape(total // 16, 16).T  # [16, T/16]
        for k in range(8):
            i16[16 * k:16 * (k + 1)] = base
        dw = DW.astype(np.float16).reshape(total // CHUNK, CHUNK).T  # [128, T/128]
        idx16_list.append(i16)
        dw_list.append(np.ascontiguousarray(dw))

    n_win = (shard + WIN - 1) // WIN
    nsub_per_win = WIN // SUB
    visits = []
    for q in range(n_tables):
        for w in range(n_win):
            lo = w * nsub_per_win
            hi = min(lo + nsub_per_win, n_sub_tot)
            ks = [int(Kmax[q, sg]) for sg in range(lo, hi)]
            visits.append((q, w, ks))

    gathers = []
    for q in range(n_tables):
        q_lo = int(starts[q, 0])
        q_hi = int(starts[q, n_sub_tot - 1] + slots_per_group[q, n_sub_tot - 1])
        p = q_lo
        while p < q_hi:
            n = min(GMAX, q_hi - p)
            gathers.append((q, p, n))
            p += n
    return dict(idx16=idx16_list, dw=dw_list, visits=visits, gathers=gathers,
                total=total)


def _host_prep(cfg, inputs):
    f32 = np.float32
    x_src = np.asarray(inputs["x_src"], f32)
    x_dst = np.asarray(inputs["x_dst"], f32)
    src_idx = np.asarray(inputs["src_idx"]).astype(np.int64)
    dst_idx = np.asarray(inputs["dst_idx"]).astype(np.int64)
    L = np.asarray(inputs["W_msg_sd"]).shape[0]

    ns, nd = x_src.shape[0], x_dst.shape[0]
    xs = np.zeros((cfg.NS_PAD, D), f32)
    xs[:ns] = x_src
    xd = np.zeros((cfg.ND_PAD, D), f32)
    xd[:nd] = x_dst

    plan_sd = _build_plan(src_idx, dst_idx, cfg.TAB_S, cfg.N_TAB_S, cfg.DST_SH)
    plan_ds = _build_plan(dst_idx, src_idx, cfg.TAB_D, cfg.N_TAB_D, cfg.SRC_SH)

    def stack_wb(wk, bk):
        w = np.asarray(inputs[wk], f32)
        b = np.asarray(inputs[bk], f32)
        out = np.empty((L, D + 1, D), np.float16)
        out[:, :D] = w.astype(np.float16)
        out[:, D] = b.astype(np.float16)
        return out

    host = dict(
        L=L,
        Wbm_sd=stack_wb("W_msg_sd", "b_msg_sd"),
        Wbm_ds=stack_wb("W_msg_ds", "b_msg_ds"),
        Wu_dst=np.asarray(inputs["W_upd_dst"], f32).astype(np.float16),
        Wu_src=np.asarray(inputs["W_upd_src"], f32).astype(np.float16),
        bu_dst=np.asarray(inputs["b_upd_dst"], f32)[:, :, None],
        bu_src=np.asarray(inputs["b_upd_src"], f32)[:, :, None],
        Win_src=np.asarray(inputs["W_in_src"], f32),
        Win_dst=np.asarray(inputs["W_in_dst"], f32),
        bin_src=np.asarray(inputs["b_in_src"], f32)[:, None],
        bin_dst=np.asarray(inputs["b_in_dst"], f32)[:, None],
        iota=np.tile(np.arange(SUB, dtype=np.float16), (128, 1)),
        xsT=[np.ascontiguousarray(xs[c * cfg.SRC_SH:(c + 1) * cfg.SRC_SH].T)
             for c in range(N_CORES)],
        xdT=[np.ascontiguousarray(xd[c * cfg.DST_SH:(c + 1) * cfg.DST_SH].T)
             for c in range(N_CORES)],
        plan_sd=plan_sd, plan_ds=plan_ds,
    )
    return host


def _build_nc(cfg, host, reps=1):
    import concourse.bass as bass
    import concourse.tile as tile
    from concourse import bacc, mybir

    dt = mybir.dt
    L = host["L"]
    plan_sd, plan_ds = host["plan_sd"], host["plan_ds"]
    TOT_SD, TOT_DS = plan_sd["total"], plan_ds["total"]

    nc = bacc.Bacc("TRN2", target_bir_lowering=False, debug=False,
                   num_devices=N_CORES, num_swdge_queues=4,
                   dynamic_dma_scratch_size=32768)

    def inp(name, shape, dtype):
        return nc.dram_tensor(name, shape, dtype, kind="ExternalInput").ap()

    xT_src = inp("xT_src", [D, cfg.SRC_SH], dt.float32)
    xT_dst = inp("xT_dst", [D, cfg.DST_SH], dt.float32)
    Win_src = inp("Win_src", [D, D], dt.float32)
    Win_dst = inp("Win_dst", [D, D], dt.float32)
    bin_src = inp("bin_src", [D, 1], dt.float32)
    bin_dst = inp("bin_dst", [D, 1], dt.float32)
    Wbm_sd = inp("Wbm_sd", [L, D + 1, D], dt.float16)
    Wbm_ds = inp("Wbm_ds", [L, D + 1, D], dt.float16)
    Wu_dst = inp("Wu_dst", [L, 2 * D, D], dt.float16)
    Wu_src = inp("Wu_src", [L, 2 * D, D], dt.float16)
    bu_dst = inp("bu_dst", [L, D, 1], dt.float32)
    bu_src = inp("bu_src", [L, D, 1], dt.float32)
    iota_in = inp("iota", [128, SUB], dt.float16)
    idx_sd = inp("idx_sd", [128, TOT_SD // 16], dt.int16)
    dw_sd = inp("dw_sd", [128, TOT_SD // CHUNK], dt.float16)
    idx_ds = inp("idx_ds", [128, TOT_DS // 16], dt.int16)
    dw_ds = inp("dw_ds", [128, TOT_DS // CHUNK], dt.float16)
    out_hd = nc.dram_tensor("out_hd", [D, cfg.DST_SH], dt.float16,
                            kind="ExternalOutput").ap()

    zs_shard = nc.dram_tensor("zs_shard", [cfg.SRC_SH, 128], dt.float16).ap()
    zd_shard = nc.dram_tensor("zd_shard", [cfg.DST_SH, 128], dt.float16).ap()
    zs_full = nc.dram_tensor("zs_full", [cfg.NS_PAD, 128], dt.float16,
                             addr_space="Shared").ap()
    zd_full = nc.dram_tensor("zd_full", [cfg.ND_PAD, 128], dt.float16,
                             addr_space="Shared").ap()

    RELU = mybir.ActivationFunctionType.Relu
    EQ = mybir.AluOpType.is_equal
    rg = [list(range(N_CORES))]

    with tile.TileContext(nc) as tc:
        from contextlib import ExitStack
        with ExitStack() as ctx:
            pers = ctx.enter_context(tc.tile_pool(name="pers", bufs=1))
            ps_agg = ctx.enter_context(
                tc.tile_pool(name="psagg", bufs=4, space="PSUM"))
            ps_mlp = ctx.enter_context(
                tc.tile_pool(name="psmlp", bufs=2, space="PSUM"))
            gath = ctx.enter_context(tc.tile_pool(name="gath", bufs=24))
            idxg = ctx.enter_context(tc.tile_pool(name="idxg", bufs=24))
            spool = ctx.enter_context(tc.tile_pool(name="spool", bufs=4))
            work = ctx.enter_context(tc.tile_pool(name="work", bufs=4))

            h_s = pers.tile([D + 1, cfg.SRC_SH], dt.float16, name="h_s")
            h_d = pers.tile([D + 1, cfg.DST_SH], dt.float16, name="h_d")
            h_t = {"s": h_s, "d": h_d}
            agg_s = pers.tile([D, cfg.SRC_SH], dt.float16)
            agg_d = pers.tile([D, cfg.DST_SH], dt.float16)
            iota_t = pers.tile([128, SUB], dt.float16)
            dw_sd_t = pers.tile([128, TOT_SD // CHUNK], dt.float16)
            dw_ds_t = pers.tile([128, TOT_DS // CHUNK], dt.float16)

            nc.sync.dma_start(out=iota_t[:], in_=iota_in[:])
            nc.sync.dma_start(out=dw_sd_t[:], in_=dw_sd[:])
            nc.sync.dma_start(out=dw_ds_t[:], in_=dw_ds[:])

            w_enc_s = pers.tile([D, D], dt.float32)
            w_enc_d = pers.tile([D, D], dt.float32)
            b_enc_s = pers.tile([D, 1], dt.float32)
            b_enc_d = pers.tile([D, 1], dt.float32)
            nc.sync.dma_start(out=w_enc_s[:], in_=Win_src[:])
            nc.sync.dma_start(out=w_enc_d[:], in_=Win_dst[:])
            nc.sync.dma_start(out=b_enc_s[:], in_=bin_src[:])
            nc.sync.dma_start(out=b_enc_d[:], in_=bin_dst[:])

            wbm_t, wu_t, bu_t = {}, {}, {}
            for l in range(L):
                for key, src in (("sd", Wbm_sd), ("ds", Wbm_ds)):
                    t = pers.tile([D + 1, D], dt.float16, name=f"wbm_{key}{l}")
                    nc.sync.dma_start(out=t[:], in_=src[l])
                    wbm_t[key, l] = t
                for key, src in (("dst", Wu_dst), ("src", Wu_src)):
                    th = pers.tile([D, D], dt.float16, name=f"wuh_{key}{l}")
                    ta = pers.tile([D, D], dt.float16, name=f"wua_{key}{l}")
                    nc.sync.dma_start(out=th[:], in_=src[l, 0:D])
                    nc.sync.dma_start(out=ta[:], in_=src[l, D:2 * D])
                    wu_t[key, l] = (th, ta)
                for key, src in (("dst", bu_dst), ("src", bu_src)):
                    t = pers.tile([D, 1], dt.float32, name=f"bu_{key}{l}")
                    nc.sync.dma_start(out=t[:], in_=src[l])
                    bu_t[key, l] = t

            for t in h_t.values():
                nc.vector.memset(t[D:D + 1, :], 1.0)

            # one-time zero fill of z-shard pad columns (never written later;
            # keeps AllGather/NaN checks clean)
            zeros64 = pers.tile([128, D], dt.float16, name="zeros64")
            nc.vector.memset(zeros64[:], 0.0)
            for z_shard, n in ((zs_shard, cfg.SRC_SH), (zd_shard, cfg.DST_SH)):
                for k in range(n // CHUNK):
                    nc.sync.dma_start(
                        out=z_shard[k * CHUNK:(k + 1) * CHUNK, D:128],
                        in_=zeros64[:])

            def encoder(xT, w_t, b_t, h_out, n):
                for j0 in range(0, n, WIN):
                    w = min(WIN, n - j0)
                    xs = work.tile([D, WIN], dt.float32, tag="xs")
                    nc.sync.dma_start(out=xs[:, :w], in_=xT[:, j0:j0 + w])
                    ps = ps_mlp.tile([D, WIN], dt.float32, tag="mlp")
                    nc.tensor.matmul(out=ps[:, :w], lhsT=w_t[:], rhs=xs[:, :w],
                                     start=True, stop=True)
                    nc.scalar.activation(out=h_out[0:D, j0:j0 + w],
                                         in_=ps[:, :w], func=RELU, bias=b_t[:])

            REPS = reps

            def z_phase(h_in, wbm, z_shard, n):
                for k in range(n // CHUNK):
                    ps = ps_mlp.tile([CHUNK, D], dt.float32, tag="mlp")
                    nc.tensor.matmul(
                        out=ps[:], lhsT=h_in[0:D + 1, k * CHUNK:(k + 1) * CHUNK],
                        rhs=wbm[:], start=True, stop=True)
                    zs = work.tile([CHUNK, D], dt.float16, tag="zstage")
                    nc.scalar.activation(out=zs[:], in_=ps[:], func=RELU)
                    nc.sync.dma_start(
                        out=z_shard[k * CHUNK:(k + 1) * CHUNK, 0:D], in_=zs[:])

            gather_count = [0]

            def sweep(plan, z_full, idx_dram, dw_t, agg_t, table_rows, shard):
                nc.vector.memset(agg_t[:], 0.0)
                gathers = plan["gathers"]
                gtiles = []
                for gno, (q, s0, nsl) in enumerate(gathers):
                    it = idxg.tile([128, nsl // 16], dt.int16, tag="idxg")
                    nc.sync.dma_start(
                        out=it[:], in_=idx_dram[:, s0 // 16:(s0 + nsl) // 16])
                    gt = gath.tile([128, (nsl // CHUNK) * 128], dt.float16,
                                   tag="gt")
                    nc.gpsimd.dma_gather(
                        gt[:].rearrange("p (b e) -> p b e", e=128),
                        z_full[q * table_rows:(q + 1) * table_rows, :],
                        it[:], nsl, nsl, 128, single_packet=False,
                        queue_num=gather_count[0] % 4)
                    gather_count[0] += 1
                    gtiles.append((s0, nsl, gt))

                SB = 8  # chunks per S-build batch
                gi = 0
                chunk0 = 0  # global chunk cursor
                s_tile = None
                for (q, w, ks) in plan["visits"]:
                    nvis = sum(ks)
                    ps = ps_agg.tile([D, WIN], dt.float32, tag="agg")
                    nc.vector.memset(ps[:], 0.0)
                    done = 0
                    for si, kk in enumerate(ks):
                        for _ in range(kk):
                            c = chunk0
                            # S batch
                            if c % SB == 0:
                                nb = min(SB, dw_t.shape[1] - c)
                                s_tile = spool.tile([128, nb * SUB], dt.float16,
                                                    tag="s")
                                nc.vector.tensor_tensor(
                                    out=s_tile[:].rearrange(
                                        "p (b e) -> p b e", e=SUB),
                                    in0=dw_t[:, c:c + nb, None].to_broadcast(
                                        [128, nb, SUB]),
                                    in1=iota_t[:, None, :].to_broadcast(
                                        [128, nb, SUB]),
                                    op=EQ)
                            # gather tile & block for this chunk
                            s0, nsl, gt = gtiles[gi]
                            if c * CHUNK >= s0 + nsl:
                                gi += 1
                                s0, nsl, gt = gtiles[gi]
                            blk = (c * CHUNK - s0) // CHUNK
                            g3 = gt[:].rearrange("p (b e) -> p b e", e=128)
                            done += 1
                            nc.tensor.matmul(
                                out=ps[:, si * SUB:(si + 1) * SUB],
                                lhsT=g3[:, blk, 0:D],
                                rhs=s_tile[:].rearrange(
                                    "p (b e) -> p b e", e=SUB)[:, c % SB, :],
                                start=False, stop=(done == nvis),
                                skip_group_check=True)
                            chunk0 += 1
                    hi = min((w + 1) * WIN, shard)
                    nc.vector.tensor_add(
                        out=agg_t[:, w * WIN:hi], in0=agg_t[:, w * WIN:hi],
                        in1=ps[:, :hi - w * WIN])

            def update(h_io, agg_t, wu, bu, n):
                # split K: ps = Wu[0:D].T @ h + Wu[D:2D].T @ agg (no concat copy)
                for j0 in range(0, n, WIN):
                    w = min(WIN, n - j0)
                    ps = ps_mlp.tile([D, WIN], dt.float32, tag="mlp")
                    nc.tensor.matmul(out=ps[:, :w], lhsT=wu[0][:],
                                     rhs=h_io[0:D, j0:j0 + w],
                                     start=True, stop=False)
                    nc.tensor.matmul(out=ps[:, :w], lhsT=wu[1][:],
                                     rhs=agg_t[:, j0:j0 + w],
                                     start=False, stop=True)
                    nc.scalar.activation(out=h_io[0:D, j0:j0 + w],
                                         in_=ps[:, :w], func=RELU, bias=bu[:])

            for _rep in range(REPS):
              encoder(xT_src, w_enc_s, b_enc_s, h_s, cfg.SRC_SH)
              encoder(xT_dst, w_enc_d, b_enc_d, h_d, cfg.DST_SH)
              for l in range(L):
                z_phase(h_s, wbm_t["sd", l], zs_shard, cfg.SRC_SH)
                nc.gpsimd.collective_compute(
                    "AllGather", mybir.AluOpType.bypass, replica_groups=rg,
                    ins=[zs_shard.opt()], outs=[zs_full.opt()])
                sweep(plan_sd, zs_full, idx_sd, dw_sd_t, agg_d,
                      cfg.TAB_S, cfg.DST_SH)
                update(h_d, agg_d, wu_t["dst", l], bu_t["dst", l], cfg.DST_SH)

                if l == L - 1:
                    nc.sync.dma_start(out=out_hd[:], in_=h_d[0:D, :])
                    break

                z_phase(h_d, wbm_t["ds", l], zd_shard, cfg.DST_SH)
                nc.gpsimd.collective_compute(
                    "AllGather", mybir.AluOpType.bypass, replica_groups=rg,
                    ins=[zd_shard.opt()], outs=[zd_full.opt()])
                sweep(plan_ds, zd_full, idx_ds, dw_ds_t, agg_s,
                      cfg.TAB_D, cfg.SRC_SH)
                update(h_s, agg_s, wu_t["src", l], bu_t["src", l], cfg.SRC_SH)

    nc.compile()
    return nc


def make_in_maps(cfg, host):
    shared = dict(
        Win_src=host["Win_src"], Win_dst=host["Win_dst"],
        bin_src=host["bin_src"], bin_dst=host["bin_dst"],
        Wbm_sd=host["Wbm_sd"], Wbm_ds=host["Wbm_ds"],
        Wu_dst=host["Wu_dst"], Wu_src=host["Wu_src"],
        bu_dst=host["bu_dst"], bu_src=host["bu_src"],
        iota=host["iota"],
    )
    maps = []
    for c in range(N_CORES):
        m = dict(shared)
        m["xT_src"] = host["xsT"][c]
        m["xT_dst"] = host["xdT"][c]
        m["idx_sd"] = host["plan_sd"]["idx16"][c]
        m["dw_sd"] = host["plan_sd"]["dw"][c]
        m["idx_ds"] = host["plan_ds"]["idx16"][c]
        m["dw_ds"] = host["plan_ds"]["dw"][c]
        maps.append(m)
    return maps


LAST_RES = None


def kernel(**inputs) -> np.ndarray:
    global LAST_RES
    cfg = REAL_CFG
    host = _host_prep(cfg, inputs)
    nc = _build_nc(cfg, host)
    from concourse.bass_utils import run_bass_kernel_spmd
    res = run_bass_kernel_spmd(nc, make_in_maps(cfg, host),
                               core_ids=list(range(N_CORES)))
    LAST_RES = res
    nd = np.asarray(inputs["x_dst"]).shape[0]
    out = np.concatenate([res.results[c]["out_hd"].T for c in range(N_CORES)],
                         axis=0)[:nd]
    return out.astype(np.float32)



# revision 19
# speedup vs baseline: 1.4415x; 1.4415x over previous
"""Bipartite GNN message passing on 8 Trainium2 NeuronCores.

Math reformulation: relu(h[idx] @ W + b) == relu(h @ W + b)[idx], so each
direction-layer is: per-node message MLP (z) -> gather z rows by edge ->
segment-sum -> update MLP.  Sharding: aggregation-side nodes are split into
8 contiguous ranges (one per core); each core owns ALL edges targeting its
range, so it computes complete aggregates locally (no AllReduce).  Only the
small per-shard z tensors are AllGathered (fp16) each direction-layer.

Segment-sum on the tensor engine: edges sorted by (gather-table, subwindow);
for each 128-edge chunk a one-hot S[e,j] = (dst_local[e] == j) matrix (built
with one DVE is_equal against a constant iota tile) turns the segment sum
into  psum[64f, 64dst] += gathered_z[128e, 64f].T @ S[128e, 64dst],
accumulated per 512-dst PSUM window and flushed additively to SBUF.

Gathers use dma_gather (one SWDGE instruction per ~8k rows, int16 indices
into <=25088-row table slices of the AllGathered z, 256B padded fp16 rows).

SPMD: one NEFF for all 8 cores, so the chunk schedule (chunks per
(table, subwindow)) is the max over cores; cores pad with (idx=0,
dst_local=sentinel) edges that contribute zero.
"""
import numpy as np

D = 64
CHUNK = 128
SUB = 64
WIN = 512
SENT = 999.0
N_CORES = 8
GMAX = 1024


class Cfg:
    def __init__(self, ns_pad, nd_pad, n_tab_s, n_tab_d):
        self.NS_PAD, self.ND_PAD = ns_pad, nd_pad
        self.SRC_SH, self.DST_SH = ns_pad // N_CORES, nd_pad // N_CORES
        self.N_TAB_S, self.N_TAB_D = n_tab_s, n_tab_d
        self.TAB_S = ns_pad // n_tab_s
        self.TAB_D = nd_pad // n_tab_d
        assert self.TAB_S <= 32767 and self.TAB_D <= 32767
        assert self.SRC_SH % CHUNK == 0 and self.DST_SH % CHUNK == 0


REAL_CFG = Cfg(100352, 50176, 4, 2)


def _build_plan(gather_idx, seg_idx, table_rows, n_tables, shard):
    """SPMD-uniform edge plan for one direction.

    Returns dict with per-core idx16 [128, TOT/16] int16, dw [128, TOT/128]
    fp16, plus uniform visits [(q, w, [k_per_sub])] and gathers
    [(q, slot0, nslots)].
    """
    n_sub_tot = shard // SUB
    core_of = seg_idx // shard
    per_core = []
    Kmax = np.zeros((n_tables, n_sub_tot), np.int64)
    for c in range(N_CORES):
        m = core_of == c
        g = gather_idx[m]
        s = seg_idx[m] - c * shard
        q = (g // table_rows).astype(np.int64)
        subg = s // SUB
        key = q * n_sub_tot + subg
        order = np.argsort(key, kind="stable")
        g, s, q, subg = g[order], s[order], q[order], subg[order]
        per_core.append((g % table_rows, s, q, subg))
        cnt = np.zeros((n_tables, n_sub_tot), np.int64)
        np.add.at(cnt, (q, subg), 1)
        Kmax = np.maximum(Kmax, (cnt + CHUNK - 1) // CHUNK)

    slots_per_group = Kmax * CHUNK
    flat = slots_per_group.reshape(-1)
    starts = np.concatenate([[0], np.cumsum(flat)[:-1]]).reshape(n_tables, n_sub_tot)
    total = int(flat.sum())
    assert total % CHUNK == 0

    idx16_list, dw_list = [], []
    for c in range(N_CORES):
        g, s, q, subg = per_core[c]
        G = np.zeros(total, np.int32)
        DW = np.full(total, SENT, np.float32)
        kk = q * n_sub_tot + subg
        bounds = np.flatnonzero(np.diff(kk)) + 1
        for grp in np.split(np.arange(len(g)), bounds):
            if len(grp) == 0:
                continue
            qq, sg = int(q[grp[0]]), int(subg[grp[0]])
            st = int(starts[qq, sg])
            n = len(grp)
            G[st:st + n] = g[grp]
            DW[st:st + n] = s[grp] - sg * SUB
        # idx16: j -> [16k + j%16, j//16] replicated for 8 gpsimd cores
        i16 = np.empty((128, total // 16), np.int16)
        base = G.astype(np.int16).reshape(total // 16, 16).T  # [16, T/16]
        for k in range(8):
            i16[16 * k:16 * (k + 1)] = base
        dw = DW.astype(np.float16).reshape(total // CHUNK, CHUNK).T  # [128, T/128]
        idx16_list.append(i16)
        dw_list.append(np.ascontiguousarray(dw))

    n_win = (shard + WIN - 1) // WIN
    nsub_per_win = WIN // SUB
    visits = []
    for q in range(n_tables):
        for w in range(n_win):
            lo = w * nsub_per_win
            hi = min(lo + nsub_per_win, n_sub_tot)
            ks = [int(Kmax[q, sg]) for sg in range(lo, hi)]
            visits.append((q, w, ks))

    gathers = []
    for q in range(n_tables):
        q_lo = int(starts[q, 0])
        q_hi = int(starts[q, n_sub_tot - 1] + slots_per_group[q, n_sub_tot - 1])
        p = q_lo
        while p < q_hi:
            n = min(GMAX, q_hi - p)
            gathers.append((q, p, n))
            p += n
    return dict(idx16=idx16_list, dw=dw_list, visits=visits, gathers=gathers,
                total=total)


def _host_prep(cfg, inputs):
    f32 = np.float32
    x_src = np.asarray(inputs["x_src"], f32)
    x_dst = np.asarray(inputs["x_dst"], f32)
    src_idx = np.asarray(inputs["src_idx"]).astype(np.int64)
    dst_idx = np.asarray(inputs["dst_idx"]).astype(np.int64)
    L = np.asarray(inputs["W_msg_sd"]).shape[0]

    ns, nd = x_src.shape[0], x_dst.shape[0]
    xs = np.zeros((cfg.NS_PAD, D), f32)
    xs[:ns] = x_src
    xd = np.zeros((cfg.ND_PAD, D), f32)
    xd[:nd] = x_dst

    plan_sd = _build_plan(src_idx, dst_idx, cfg.TAB_S, cfg.N_TAB_S, cfg.DST_SH)
    plan_ds = _build_plan(dst_idx, src_idx, cfg.TAB_D, cfg.N_TAB_D, cfg.SRC_SH)

    def stack_wb(wk, bk):
        w = np.asarray(inputs[wk], f32)
        b = np.asarray(inputs[bk], f32)
        out = np.empty((L, D + 1, D), np.float16)
        out[:, :D] = w.astype(np.float16)
        out[:, D] = b.astype(np.float16)
        return out

    host = dict(
        L=L,
        Wbm_sd=stack_wb("W_msg_sd", "b_msg_sd"),
        Wbm_ds=stack_wb("W_msg_ds", "b_msg_ds"),
        Wu_dst=np.asarray(inputs["W_upd_dst"], f32).astype(np.float16),
        Wu_src=np.asarray(inputs["W_upd_src"], f32).astype(np.float16),
        bu_dst=np.asarray(inputs["b_upd_dst"], f32)[:, :, None],
        bu_src=np.asarray(inputs["b_upd_src"], f32)[:, :, None],
        Win_src=np.asarray(inputs["W_in_src"], f32),
        Win_dst=np.asarray(inputs["W_in_dst"], f32),
        bin_src=np.asarray(inputs["b_in_src"], f32)[:, None],
        bin_dst=np.asarray(inputs["b_in_dst"], f32)[:, None],
        iota=np.tile(np.arange(SUB, dtype=np.float16), (128, 1)),
        xsT=[np.ascontiguousarray(xs[c * cfg.SRC_SH:(c + 1) * cfg.SRC_SH].T)
             for c in range(N_CORES)],
        xdT=[np.ascontiguousarray(xd[c * cfg.DST_SH:(c + 1) * cfg.DST_SH].T)
             for c in range(N_CORES)],
        plan_sd=plan_sd, plan_ds=plan_ds,
    )
    return host


def _build_nc(cfg, host, reps=1):
    import concourse.bass as bass
    import concourse.tile as tile
    from concourse import bacc, mybir

    dt = mybir.dt
    L = host["L"]
    plan_sd, plan_ds = host["plan_sd"], host["plan_ds"]
    TOT_SD, TOT_DS = plan_sd["total"], plan_ds["total"]

    nc = bacc.Bacc("TRN2", target_bir_lowering=False, debug=False,
                   num_devices=N_CORES, num_swdge_queues=4,
                   dynamic_dma_scratch_size=32768)

    def inp(name, shape, dtype):
        return nc.dram_tensor(name, shape, dtype, kind="ExternalInput").ap()

    xT_src = inp("xT_src", [D, cfg.SRC_SH], dt.float32)
    xT_dst = inp("xT_dst", [D, cfg.DST_SH], dt.float32)
    Win_src = inp("Win_src", [D, D], dt.float32)
    Win_dst = inp("Win_dst", [D, D], dt.float32)
    bin_src = inp("bin_src", [D, 1], dt.float32)
    bin_dst = inp("bin_dst", [D, 1], dt.float32)
    Wbm_sd = inp("Wbm_sd", [L, D + 1, D], dt.float16)
    Wbm_ds = inp("Wbm_ds", [L, D + 1, D], dt.float16)
    Wu_dst = inp("Wu_dst", [L, 2 * D, D], dt.float16)
    Wu_src = inp("Wu_src", [L, 2 * D, D], dt.float16)
    bu_dst = inp("bu_dst", [L, D, 1], dt.float32)
    bu_src = inp("bu_src", [L, D, 1], dt.float32)
    iota_in = inp("iota", [128, SUB], dt.float16)
    idx_sd = inp("idx_sd", [128, TOT_SD // 16], dt.int16)
    dw_sd = inp("dw_sd", [128, TOT_SD // CHUNK], dt.float16)
    idx_ds = inp("idx_ds", [128, TOT_DS // 16], dt.int16)
    dw_ds = inp("dw_ds", [128, TOT_DS // CHUNK], dt.float16)
    out_hd = nc.dram_tensor("out_hd", [D, cfg.DST_SH], dt.float16,
                            kind="ExternalOutput").ap()

    zs_shard = nc.dram_tensor("zs_shard", [cfg.SRC_SH, 128], dt.float16).ap()
    zd_shard = nc.dram_tensor("zd_shard", [cfg.DST_SH, 128], dt.float16).ap()
    zs_full = nc.dram_tensor("zs_full", [cfg.NS_PAD, 128], dt.float16,
                             addr_space="Shared").ap()
    zd_full = nc.dram_tensor("zd_full", [cfg.ND_PAD, 128], dt.float16,
                             addr_space="Shared").ap()
    # gathers from Shared-space DRAM are ~1.8x slower per descriptor than
    # local DRAM; copy each table locally after the AllGather (per-table
    # tensors so table-q gathers only wait on table-q's copy)
    zs_loc = [nc.dram_tensor(f"zs_loc{q}", [cfg.TAB_S, 128], dt.float16).ap()
              for q in range(cfg.N_TAB_S)]
    zd_loc = [nc.dram_tensor(f"zd_loc{q}", [cfg.TAB_D, 128], dt.float16).ap()
              for q in range(cfg.N_TAB_D)]

    RELU = mybir.ActivationFunctionType.Relu
    EQ = mybir.AluOpType.is_equal
    rg = [list(range(N_CORES))]

    with tile.TileContext(nc) as tc:
        from contextlib import ExitStack
        with ExitStack() as ctx:
            pers = ctx.enter_context(tc.tile_pool(name="pers", bufs=1))
            ps_agg = ctx.enter_context(
                tc.tile_pool(name="psagg", bufs=4, space="PSUM"))
            ps_mlp = ctx.enter_context(
                tc.tile_pool(name="psmlp", bufs=2, space="PSUM"))
            gath = ctx.enter_context(tc.tile_pool(name="gath", bufs=24))
            idxg = ctx.enter_context(tc.tile_pool(name="idxg", bufs=24))
            spool = ctx.enter_context(tc.tile_pool(name="spool", bufs=4))
            work = ctx.enter_context(tc.tile_pool(name="work", bufs=4))

            h_s = pers.tile([D + 1, cfg.SRC_SH], dt.float16, name="h_s")
            h_d = pers.tile([D + 1, cfg.DST_SH], dt.float16, name="h_d")
            h_t = {"s": h_s, "d": h_d}
            agg_s = pers.tile([D, cfg.SRC_SH], dt.float16)
            agg_d = pers.tile([D, cfg.DST_SH], dt.float16)
            iota_t = pers.tile([128, SUB], dt.float16)
            dw_sd_t = pers.tile([128, TOT_SD // CHUNK], dt.float16)
            dw_ds_t = pers.tile([128, TOT_DS // CHUNK], dt.float16)

            nc.sync.dma_start(out=iota_t[:], in_=iota_in[:])
            nc.sync.dma_start(out=dw_sd_t[:], in_=dw_sd[:])
            nc.sync.dma_start(out=dw_ds_t[:], in_=dw_ds[:])

            w_enc_s = pers.tile([D, D], dt.float32)
            w_enc_d = pers.tile([D, D], dt.float32)
            b_enc_s = pers.tile([D, 1], dt.float32)
            b_enc_d = pers.tile([D, 1], dt.float32)
            nc.sync.dma_start(out=w_enc_s[:], in_=Win_src[:])
            nc.sync.dma_start(out=w_enc_d[:], in_=Win_dst[:])
            nc.sync.dma_start(out=b_enc_s[:], in_=bin_src[:])
            nc.sync.dma_start(out=b_enc_d[:], in_=bin_dst[:])

            wbm_t, wu_t, bu_t = {}, {}, {}
            for l in range(L):
                for key, src in (("sd", Wbm_sd), ("ds", Wbm_ds)):
                    t = pers.tile([D + 1, D], dt.float16, name=f"wbm_{key}{l}")
                    nc.sync.dma_start(out=t[:], in_=src[l])
                    wbm_t[key, l] = t
                for key, src in (("dst", Wu_dst), ("src", Wu_src)):
                    th = pers.tile([D, D], dt.float16, name=f"wuh_{key}{l}")
                    ta = pers.tile([D, D], dt.float16, name=f"wua_{key}{l}")
                    nc.sync.dma_start(out=th[:], in_=src[l, 0:D])
                    nc.sync.dma_start(out=ta[:], in_=src[l, D:2 * D])
                    wu_t[key, l] = (th, ta)
                for key, src in (("dst", bu_dst), ("src", bu_src)):
                    t = pers.tile([D, 1], dt.float32, name=f"bu_{key}{l}")
                    nc.sync.dma_start(out=t[:], in_=src[l])
                    bu_t[key, l] = t

            for t in h_t.values():
                nc.vector.memset(t[D:D + 1, :], 1.0)

            # one-time zero fill of z-shard pad columns (never written later;
            # keeps AllGather/NaN checks clean)
            zeros64 = pers.tile([128, D], dt.float16, name="zeros64")
            nc.vector.memset(zeros64[:], 0.0)
            for z_shard, n in ((zs_shard, cfg.SRC_SH), (zd_shard, cfg.DST_SH)):
                for k in range(n // CHUNK):
                    nc.sync.dma_start(
                        out=z_shard[k * CHUNK:(k + 1) * CHUNK, D:128],
                        in_=zeros64[:])

            def encoder(xT, w_t, b_t, h_out, n):
                for j0 in range(0, n, WIN):
                    w = min(WIN, n - j0)
                    xs = work.tile([D, WIN], dt.float32, tag="xs")
                    nc.sync.dma_start(out=xs[:, :w], in_=xT[:, j0:j0 + w])
                    ps = ps_mlp.tile([D, WIN], dt.float32, tag="mlp")
                    nc.tensor.matmul(out=ps[:, :w], lhsT=w_t[:], rhs=xs[:, :w],
                                     start=True, stop=True)
                    nc.scalar.activation(out=h_out[0:D, j0:j0 + w],
                                         in_=ps[:, :w], func=RELU, bias=b_t[:])

            REPS = reps

            def z_phase(h_in, wbm, z_shard, n):
                for k in range(n // CHUNK):
                    ps = ps_mlp.tile([CHUNK, D], dt.float32, tag="mlp")
                    nc.tensor.matmul(
                        out=ps[:], lhsT=h_in[0:D + 1, k * CHUNK:(k + 1) * CHUNK],
                        rhs=wbm[:], start=True, stop=True)
                    zs = work.tile([CHUNK, D], dt.float16, tag="zstage")
                    nc.scalar.activation(out=zs[:], in_=ps[:], func=RELU)
                    nc.sync.dma_start(
                        out=z_shard[k * CHUNK:(k + 1) * CHUNK, 0:D], in_=zs[:])

            gather_count = [0]

            def sweep(plan, z_tabs, idx_dram, dw_t, agg_t, table_rows, shard):
                nc.vector.memset(agg_t[:], 0.0)
                gathers = plan["gathers"]
                gtiles = []
                for gno, (q, s0, nsl) in enumerate(gathers):
                    it = idxg.tile([128, nsl // 16], dt.int16, tag="idxg")
                    nc.sync.dma_start(
                        out=it[:], in_=idx_dram[:, s0 // 16:(s0 + nsl) // 16])
                    gt = gath.tile([128, (nsl // CHUNK) * 128], dt.float16,
                                   tag="gt")
                    nc.gpsimd.dma_gather(
                        gt[:].rearrange("p (b e) -> p b e", e=128),
                        z_tabs[q][:],
                        it[:], nsl, nsl, 128, single_packet=False,
                        queue_num=gather_count[0] % 4)
                    gather_count[0] += 1
                    gtiles.append((s0, nsl, gt))

                SB = 8  # chunks per S-build batch
                gi = 0
                chunk0 = 0  # global chunk cursor
                s_tile = None
                for (q, w, ks) in plan["visits"]:
                    nvis = sum(ks)
                    ps = ps_agg.tile([D, WIN], dt.float32, tag="agg")
                    nc.vector.memset(ps[:], 0.0)
                    done = 0
                    for si, kk in enumerate(ks):
                        for _ in range(kk):
                            c = chunk0
                            # S batch
                            if c % SB == 0:
                                nb = min(SB, dw_t.shape[1] - c)
                                s_tile = spool.tile([128, nb * SUB], dt.float16,
                                                    tag="s")
                                nc.vector.tensor_tensor(
                                    out=s_tile[:].rearrange(
                                        "p (b e) -> p b e", e=SUB),
                                    in0=dw_t[:, c:c + nb, None].to_broadcast(
                                        [128, nb, SUB]),
                                    in1=iota_t[:, None, :].to_broadcast(
                                        [128, nb, SUB]),
                                    op=EQ)
                            # gather tile & block for this chunk
                            s0, nsl, gt = gtiles[gi]
                            if c * CHUNK >= s0 + nsl:
                                gi += 1
                                s0, nsl, gt = gtiles[gi]
                            blk = (c * CHUNK - s0) // CHUNK
                            g3 = gt[:].rearrange("p (b e) -> p b e", e=128)
                            done += 1
                            nc.tensor.matmul(
                                out=ps[:, si * SUB:(si + 1) * SUB],
                                lhsT=g3[:, blk, 0:D],
                                rhs=s_tile[:].rearrange(
                                    "p (b e) -> p b e", e=SUB)[:, c % SB, :],
                                start=False, stop=(done == nvis),
                                skip_group_check=True)
                            chunk0 += 1
                    hi = min((w + 1) * WIN, shard)
                    nc.vector.tensor_add(
                        out=agg_t[:, w * WIN:hi], in0=agg_t[:, w * WIN:hi],
                        in1=ps[:, :hi - w * WIN])

            def update(h_io, agg_t, wu, bu, n):
                # split K: ps = Wu[0:D].T @ h + Wu[D:2D].T @ agg (no concat copy)
                for j0 in range(0, n, WIN):
                    w = min(WIN, n - j0)
                    ps = ps_mlp.tile([D, WIN], dt.float32, tag="mlp")
                    nc.tensor.matmul(out=ps[:, :w], lhsT=wu[0][:],
                                     rhs=h_io[0:D, j0:j0 + w],
                                     start=True, stop=False)
                    nc.tensor.matmul(out=ps[:, :w], lhsT=wu[1][:],
                                     rhs=agg_t[:, j0:j0 + w],
                                     start=False, stop=True)
                    nc.scalar.activation(out=h_io[0:D, j0:j0 + w],
                                         in_=ps[:, :w], func=RELU, bias=bu[:])

            for _rep in range(REPS):
              encoder(xT_src, w_enc_s, b_enc_s, h_s, cfg.SRC_SH)
              encoder(xT_dst, w_enc_d, b_enc_d, h_d, cfg.DST_SH)
              for l in range(L):
                z_phase(h_s, wbm_t["sd", l], zs_shard, cfg.SRC_SH)
                nc.gpsimd.collective_compute(
                    "AllGather", mybir.AluOpType.bypass, replica_groups=rg,
                    ins=[zs_shard.opt()], outs=[zs_full.opt()])
                for q in range(cfg.N_TAB_S):
                    nc.sync.dma_start(
                        out=zs_loc[q][:],
                        in_=zs_full[q * cfg.TAB_S:(q + 1) * cfg.TAB_S, :])
                sweep(plan_sd, zs_loc, idx_sd, dw_sd_t, agg_d,
                      cfg.TAB_S, cfg.DST_SH)
                update(h_d, agg_d, wu_t["dst", l], bu_t["dst", l], cfg.DST_SH)

                if l == L - 1:
                    nc.sync.dma_start(out=out_hd[:], in_=h_d[0:D, :])
                    break

                z_phase(h_d, wbm_t["ds", l], zd_shard, cfg.DST_SH)
                nc.gpsimd.collective_compute(
                    "AllGather", mybir.AluOpType.bypass, replica_groups=rg,
                    ins=[zd_shard.opt()], outs=[zd_full.opt()])
                for q in range(cfg.N_TAB_D):
                    nc.sync.dma_start(
                        out=zd_loc[q][:],
                        in_=zd_full[q * cfg.TAB_D:(q + 1) * cfg.TAB_D, :])
                sweep(plan_ds, zd_loc, idx_ds, dw_ds_t, agg_s,
                      cfg.TAB_D, cfg.SRC_SH)
                update(h_s, agg_s, wu_t["src", l], bu_t["src", l], cfg.SRC_SH)

    nc.compile()
    return nc


def make_in_maps(cfg, host):
    shared = dict(
        Win_src=host["Win_src"], Win_dst=host["Win_dst"],
        bin_src=host["bin_src"], bin_dst=host["bin_dst"],
        Wbm_sd=host["Wbm_sd"], Wbm_ds=host["Wbm_ds"],
        Wu_dst=host["Wu_dst"], Wu_src=host["Wu_src"],
        bu_dst=host["bu_dst"], bu_src=host["bu_src"],
        iota=host["iota"],
    )
    maps = []
    for c in range(N_CORES):
        m = dict(shared)
        m["xT_src"] = host["xsT"][c]
        m["xT_dst"] = host["xdT"][c]
        m["idx_sd"] = host["plan_sd"]["idx16"][c]
        m["dw_sd"] = host["plan_sd"]["dw"][c]
        m["idx_ds"] = host["plan_ds"]["idx16"][c]
        m["dw_ds"] = host["plan_ds"]["dw"][c]
        maps.append(m)
    return maps


LAST_RES = None


def kernel(**inputs) -> np.ndarray:
    global LAST_RES
    cfg = REAL_CFG
    host = _host_prep(cfg, inputs)
    nc = _build_nc(cfg, host)
    from concourse.bass_utils import run_bass_kernel_spmd
    res = run_bass_kernel_spmd(nc, make_in_maps(cfg, host),
                               core_ids=list(range(N_CORES)))
    LAST_RES = res
    nd = np.asarray(inputs["x_dst"]).shape[0]
    out = np.concatenate([res.results[c]["out_hd"].T for c in range(N_CORES)],
                         axis=0)[:nd]
    return out.astype(np.float32)



# revision 25
# speedup vs baseline: 1.6821x; 1.1669x over previous
"""Bipartite GNN message passing on 8 Trainium2 NeuronCores.

Math reformulation: relu(h[idx] @ W + b) == relu(h @ W + b)[idx], so each
direction-layer is: per-node message MLP (z) -> gather z rows by edge ->
segment-sum -> update MLP.  Sharding: aggregation-side nodes are split into
8 contiguous ranges (one per core); each core owns ALL edges targeting its
range, so it computes complete aggregates locally (no AllReduce).  Only the
small per-shard z tensors are AllGathered (fp16) each direction-layer.

Segment-sum on the tensor engine: edges sorted by (gather-table, subwindow);
for each 128-edge chunk a one-hot S[e,j] = (dst_local[e] == j) matrix (built
with one DVE is_equal against a constant iota tile) turns the segment sum
into  psum[64f, 64dst] += gathered_z[128e, 64f].T @ S[128e, 64dst],
accumulated per 512-dst PSUM window and flushed additively to SBUF.

Gathers use dma_gather (one SWDGE instruction per ~8k rows, int16 indices
into <=25088-row table slices of the AllGathered z, 256B padded fp16 rows).

SPMD: one NEFF for all 8 cores, so the chunk schedule (chunks per
(table, subwindow)) is the max over cores; cores pad with (idx=0,
dst_local=sentinel) edges that contribute zero.
"""
import numpy as np

D = 64
CHUNK = 128
SUB = 64
WIN = 512
SENT = 999.0
N_CORES = 8
GMAX = 1024


class Cfg:
    def __init__(self, ns_pad, nd_pad, n_tab_s, n_tab_d):
        self.NS_PAD, self.ND_PAD = ns_pad, nd_pad
        self.SRC_SH, self.DST_SH = ns_pad // N_CORES, nd_pad // N_CORES
        self.N_TAB_S, self.N_TAB_D = n_tab_s, n_tab_d
        self.TAB_S = ns_pad // n_tab_s
        self.TAB_D = nd_pad // n_tab_d
        assert self.TAB_S <= 32767 and self.TAB_D <= 32767
        assert self.SRC_SH % CHUNK == 0 and self.DST_SH % CHUNK == 0


REAL_CFG = Cfg(100352, 50176, 4, 2)


def _build_plan(gather_idx, seg_idx, table_rows, n_tables, shard):
    """SPMD-uniform edge plan for one direction.

    Returns dict with per-core idx16 [128, TOT/16] int16, dw [128, TOT/128]
    fp16, plus uniform visits [(q, w, [k_per_sub])] and gathers
    [(q, slot0, nslots)].
    """
    n_sub_tot = shard // SUB
    core_of = seg_idx // shard
    per_core = []
    Kmax = np.zeros((n_tables, n_sub_tot), np.int64)
    for c in range(N_CORES):
        m = core_of == c
        g = gather_idx[m]
        s = seg_idx[m] - c * shard
        q = (g // table_rows).astype(np.int64)
        subg = s // SUB
        key = q * n_sub_tot + subg
        order = np.argsort(key, kind="stable")
        g, s, q, subg = g[order], s[order], q[order], subg[order]
        per_core.append((g % table_rows, s, q, subg))
        cnt = np.zeros((n_tables, n_sub_tot), np.int64)
        np.add.at(cnt, (q, subg), 1)
        Kmax = np.maximum(Kmax, (cnt + CHUNK - 1) // CHUNK)

    slots_per_group = Kmax * CHUNK
    flat = slots_per_group.reshape(-1)
    starts = np.concatenate([[0], np.cumsum(flat)[:-1]]).reshape(n_tables, n_sub_tot)
    total = int(flat.sum())
    assert total % CHUNK == 0

    idx16_list, dw_list = [], []
    for c in range(N_CORES):
        g, s, q, subg = per_core[c]
        G = np.zeros(total, np.int32)
        DW = np.full(total, SENT, np.float32)
        kk = q * n_sub_tot + subg
        bounds = np.flatnonzero(np.diff(kk)) + 1
        for grp in np.split(np.arange(len(g)), bounds):
            if len(grp) == 0:
                continue
            qq, sg = int(q[grp[0]]), int(subg[grp[0]])
            st = int(starts[qq, sg])
            n = len(grp)
            G[st:st + n] = g[grp]
            DW[st:st + n] = s[grp] - sg * SUB
        # idx16: j -> [16k + j%16, j//16] replicated for 8 gpsimd cores
        i16 = np.empty((128, total // 16), np.int16)
        base = G.astype(np.int16).reshape(total // 16, 16).T  # [16, T/16]
        for k in range(8):
            i16[16 * k:16 * (k + 1)] = base
        dw = DW.astype(np.float16).reshape(total // CHUNK, CHUNK).T  # [128, T/128]
        idx16_list.append(i16)
        dw_list.append(np.ascontiguousarray(dw))

    n_win = (shard + WIN - 1) // WIN
    nsub_per_win = WIN // SUB
    visits = []
    for q in range(n_tables):
        for w in range(n_win):
            lo = w * nsub_per_win
            hi = min(lo + nsub_per_win, n_sub_tot)
            ks = [int(Kmax[q, sg]) for sg in range(lo, hi)]
            visits.append((q, w, ks))

    gathers = []
    for q in range(n_tables):
        q_lo = int(starts[q, 0])
        q_hi = int(starts[q, n_sub_tot - 1] + slots_per_group[q, n_sub_tot - 1])
        p = q_lo
        while p < q_hi:
            n = min(GMAX, q_hi - p)
            gathers.append((q, p, n))
            p += n
    return dict(idx16=idx16_list, dw=dw_list, visits=visits, gathers=gathers,
                total=total)


def _balance_relabel(gather_idx, seg_idx, table_rows, n_tables, shard):
    """Per-core node relabeling within each seg-side shard so per
    (gather-table, 64-sub) edge counts stay <= 4*CHUNK for all "hard" subs,
    with excess concentrated in the tail-window "overflow" subs (same subs
    on every core, so the SPMD Kmax stays 4 for hard groups).
    Returns perm[NCORES, shard]: perm[c, old] = new."""
    n_sub = shard // SUB
    n_over = n_sub - 8 * (n_sub // 8)  # tail-window subs
    if n_over == 0:
        n_over = 2
    n_hard = n_sub - n_over
    target = 4 * CHUNK
    core_of = seg_idx // shard
    q_of = gather_idx // table_rows
    perm = np.empty((N_CORES, shard), np.int64)
    for c in range(N_CORES):
        m = core_of == c
        loc = seg_idx[m] - c * shard
        qq = q_of[m]
        deg = np.zeros((shard, n_tables), np.int64)
        np.add.at(deg, (loc, qq), 1)
        order = np.argsort(-deg.sum(1), kind="stable")
        cnt = np.zeros((n_sub, n_tables), np.int64)
        cap = np.full(n_sub, SUB, np.int64)
        fill_pos = np.zeros(n_sub, np.int64)

        def place(node, b):
            perm[c, node] = b * SUB + fill_pos[b]
            fill_pos[b] += 1
            cnt[b] += deg[node]
            cap[b] -= 1

        # heaviest nodes -> overflow subs (they absorb the excess over the
        # 512 hard cap; overflow groups may have Kmax 5-8, that's fine)
        T = n_over * SUB
        for node in order[:T]:
            ov = np.arange(n_hard, n_sub)
            ovc = ov[cap[ov] > 0]
            b = int(ovc[np.argmin((cnt[ovc] + deg[node]).max(1))])
            place(node, b)
        # rest: best-fit into hard bins under the cap
        for node in order[T:]:
            d = deg[node]
            nc_ = cnt[:n_hard] + d
            feas = (nc_ <= target).all(1) & (cap[:n_hard] > 0)
            if feas.any():
                tot = nc_.sum(1)
                tot[~feas] = -1
                b = int(np.argmax(tot))
            else:
                sp = np.maximum(cnt + d - target, 0).sum(1)
                sp[cap == 0] = 1 << 40
                b = int(np.argmin(sp))
            place(node, b)
        _swap_repair(deg, perm[c], cnt, n_hard, target)
    return perm


def _swap_repair(deg, perm_c, cnt, n_hard, target, rounds=8):
    """Node-for-node swaps to push over-cap hard (bin, q) groups under
    `target`. Swapping perm values preserves bin capacities exactly."""
    bin_of = perm_c // SUB
    for _ in range(rounds):
        viol = np.argwhere(cnt[:n_hard] > target)
        if not len(viol):
            break
        fixed_any = False
        for b, q in viol:
            guard = 0
            while cnt[b, q] > target and guard < 16:
                guard += 1
                nodes_b = np.where(bin_of == b)[0]
                x = nodes_b[np.argmax(deg[nodes_b, q])]
                dx = deg[x]
                order_u = np.argsort(cnt[:n_hard, q])
                done = False
                for u in order_u[:24]:
                    if u == b:
                        continue
                    nodes_u = np.where(bin_of == u)[0]
                    need = np.maximum(cnt[u] + dx - target, 0)
                    ub = target - cnt[b] + dx
                    dnu = deg[nodes_u]
                    ok = ((dnu >= need).all(1) & (dnu <= ub).all(1)
                          & (dnu[:, q] < dx[q]))
                    cand = nodes_u[ok]
                    if len(cand):
                        y = cand[np.argmin(deg[cand].sum(1))]
                        bin_of[x], bin_of[y] = u, b
                        perm_c[x], perm_c[y] = perm_c[y], perm_c[x]
                        cnt[b] += deg[y] - dx
                        cnt[u] += dx - deg[y]
                        done = fixed_any = True
                        break
                if not done:
                    break
        if not fixed_any:
            break


def _host_prep(cfg, inputs):
    f32 = np.float32
    x_src = np.asarray(inputs["x_src"], f32)
    x_dst = np.asarray(inputs["x_dst"], f32)
    src_idx = np.asarray(inputs["src_idx"]).astype(np.int64)
    dst_idx = np.asarray(inputs["dst_idx"]).astype(np.int64)
    L = np.asarray(inputs["W_msg_sd"]).shape[0]

    ns, nd = x_src.shape[0], x_dst.shape[0]
    xs = np.zeros((cfg.NS_PAD, D), f32)
    xs[:ns] = x_src
    xd = np.zeros((cfg.ND_PAD, D), f32)
    xd[:nd] = x_dst

    # balance (table, sub) group counts across cores via node relabeling
    perm_d = _balance_relabel(src_idx, dst_idx, cfg.TAB_S, cfg.N_TAB_S,
                              cfg.DST_SH)
    perm_s = _balance_relabel(dst_idx, src_idx, cfg.TAB_D, cfg.N_TAB_D,
                              cfg.SRC_SH)
    dc = dst_idx // cfg.DST_SH
    dst_rel = dc * cfg.DST_SH + perm_d[dc, dst_idx % cfg.DST_SH]
    sc = src_idx // cfg.SRC_SH
    src_rel = sc * cfg.SRC_SH + perm_s[sc, src_idx % cfg.SRC_SH]
    # apply the same relabeling to node feature rows: new row k of shard c
    # holds old node argwhere(perm==k)
    inv_d = np.empty_like(perm_d)
    inv_s = np.empty_like(perm_s)
    for c in range(N_CORES):
        inv_d[c, perm_d[c]] = np.arange(cfg.DST_SH)
        inv_s[c, perm_s[c]] = np.arange(cfg.SRC_SH)
        xs[c * cfg.SRC_SH:(c + 1) * cfg.SRC_SH] = \
            xs[c * cfg.SRC_SH:(c + 1) * cfg.SRC_SH][inv_s[c]]
        xd[c * cfg.DST_SH:(c + 1) * cfg.DST_SH] = \
            xd[c * cfg.DST_SH:(c + 1) * cfg.DST_SH][inv_d[c]]

    plan_sd = _build_plan(src_rel, dst_rel, cfg.TAB_S, cfg.N_TAB_S, cfg.DST_SH)
    plan_ds = _build_plan(dst_rel, src_rel, cfg.TAB_D, cfg.N_TAB_D, cfg.SRC_SH)

    def stack_wb(wk, bk):
        w = np.asarray(inputs[wk], f32)
        b = np.asarray(inputs[bk], f32)
        out = np.empty((L, D + 1, D), np.float16)
        out[:, :D] = w.astype(np.float16)
        out[:, D] = b.astype(np.float16)
        return out

    host = dict(
        L=L,
        Wbm_sd=stack_wb("W_msg_sd", "b_msg_sd"),
        Wbm_ds=stack_wb("W_msg_ds", "b_msg_ds"),
        Wu_dst=np.asarray(inputs["W_upd_dst"], f32).astype(np.float16),
        Wu_src=np.asarray(inputs["W_upd_src"], f32).astype(np.float16),
        bu_dst=np.asarray(inputs["b_upd_dst"], f32)[:, :, None],
        bu_src=np.asarray(inputs["b_upd_src"], f32)[:, :, None],
        Win_src=np.asarray(inputs["W_in_src"], f32),
        Win_dst=np.asarray(inputs["W_in_dst"], f32),
        bin_src=np.asarray(inputs["b_in_src"], f32)[:, None],
        bin_dst=np.asarray(inputs["b_in_dst"], f32)[:, None],
        iota=np.tile(np.arange(SUB, dtype=np.float16), (128, 1)),
        xsT=[np.ascontiguousarray(xs[c * cfg.SRC_SH:(c + 1) * cfg.SRC_SH].T)
             for c in range(N_CORES)],
        xdT=[np.ascontiguousarray(xd[c * cfg.DST_SH:(c + 1) * cfg.DST_SH].T)
             for c in range(N_CORES)],
        plan_sd=plan_sd, plan_ds=plan_ds, perm_d=perm_d,
    )
    return host


def _build_nc(cfg, host, reps=1):
    import concourse.bass as bass
    import concourse.tile as tile
    from concourse import bacc, mybir

    dt = mybir.dt
    L = host["L"]
    plan_sd, plan_ds = host["plan_sd"], host["plan_ds"]
    TOT_SD, TOT_DS = plan_sd["total"], plan_ds["total"]

    nc = bacc.Bacc("TRN2", target_bir_lowering=False, debug=False,
                   num_devices=N_CORES, num_swdge_queues=4,
                   dynamic_dma_scratch_size=32768)

    def inp(name, shape, dtype):
        return nc.dram_tensor(name, shape, dtype, kind="ExternalInput").ap()

    xT_src = inp("xT_src", [D, cfg.SRC_SH], dt.float32)
    xT_dst = inp("xT_dst", [D, cfg.DST_SH], dt.float32)
    Win_src = inp("Win_src", [D, D], dt.float32)
    Win_dst = inp("Win_dst", [D, D], dt.float32)
    bin_src = inp("bin_src", [D, 1], dt.float32)
    bin_dst = inp("bin_dst", [D, 1], dt.float32)
    Wbm_sd = inp("Wbm_sd", [L, D + 1, D], dt.float16)
    Wbm_ds = inp("Wbm_ds", [L, D + 1, D], dt.float16)
    Wu_dst = inp("Wu_dst", [L, 2 * D, D], dt.float16)
    Wu_src = inp("Wu_src", [L, 2 * D, D], dt.float16)
    bu_dst = inp("bu_dst", [L, D, 1], dt.float32)
    bu_src = inp("bu_src", [L, D, 1], dt.float32)
    iota_in = inp("iota", [128, SUB], dt.float16)
    idx_sd = inp("idx_sd", [128, TOT_SD // 16], dt.int16)
    dw_sd = inp("dw_sd", [128, TOT_SD // CHUNK], dt.float16)
    idx_ds = inp("idx_ds", [128, TOT_DS // 16], dt.int16)
    dw_ds = inp("dw_ds", [128, TOT_DS // CHUNK], dt.float16)
    out_hd = nc.dram_tensor("out_hd", [D, cfg.DST_SH], dt.float16,
                            kind="ExternalOutput").ap()

    zs_shard = nc.dram_tensor("zs_shard", [cfg.SRC_SH, 128], dt.float16).ap()
    zd_shard = nc.dram_tensor("zd_shard", [cfg.DST_SH, 128], dt.float16).ap()
    zs_full = nc.dram_tensor("zs_full", [cfg.NS_PAD, 128], dt.float16,
                             addr_space="Shared").ap()
    zd_full = nc.dram_tensor("zd_full", [cfg.ND_PAD, 128], dt.float16,
                             addr_space="Shared").ap()
    # gathers from Shared-space DRAM are ~1.8x slower per descriptor than
    # local DRAM; copy each table locally after the AllGather (per-table
    # tensors so table-q gathers only wait on table-q's copy)
    zs_loc = [nc.dram_tensor(f"zs_loc{q}", [cfg.TAB_S, 128], dt.float16).ap()
              for q in range(cfg.N_TAB_S)]
    zd_loc = [nc.dram_tensor(f"zd_loc{q}", [cfg.TAB_D, 128], dt.float16).ap()
              for q in range(cfg.N_TAB_D)]

    RELU = mybir.ActivationFunctionType.Relu
    EQ = mybir.AluOpType.is_equal
    rg = [list(range(N_CORES))]

    with tile.TileContext(nc) as tc:
        from contextlib import ExitStack
        with ExitStack() as ctx:
            pers = ctx.enter_context(tc.tile_pool(name="pers", bufs=1))
            ps_agg = ctx.enter_context(
                tc.tile_pool(name="psagg", bufs=4, space="PSUM"))
            ps_mlp = ctx.enter_context(
                tc.tile_pool(name="psmlp", bufs=2, space="PSUM"))
            gath = ctx.enter_context(tc.tile_pool(name="gath", bufs=24))
            idxg = ctx.enter_context(tc.tile_pool(name="idxg", bufs=24))
            spool = ctx.enter_context(tc.tile_pool(name="spool", bufs=4))
            work = ctx.enter_context(tc.tile_pool(name="work", bufs=4))

            h_s = pers.tile([D + 1, cfg.SRC_SH], dt.float16, name="h_s")
            h_d = pers.tile([D + 1, cfg.DST_SH], dt.float16, name="h_d")
            h_t = {"s": h_s, "d": h_d}
            agg_s = pers.tile([D, cfg.SRC_SH], dt.float16)
            agg_d = pers.tile([D, cfg.DST_SH], dt.float16)
            iota_t = pers.tile([128, SUB], dt.float16)
            dw_sd_t = pers.tile([128, TOT_SD // CHUNK], dt.float16)
            dw_ds_t = pers.tile([128, TOT_DS // CHUNK], dt.float16)

            nc.sync.dma_start(out=iota_t[:], in_=iota_in[:])
            nc.sync.dma_start(out=dw_sd_t[:], in_=dw_sd[:])
            nc.sync.dma_start(out=dw_ds_t[:], in_=dw_ds[:])

            w_enc_s = pers.tile([D, D], dt.float32)
            w_enc_d = pers.tile([D, D], dt.float32)
            b_enc_s = pers.tile([D, 1], dt.float32)
            b_enc_d = pers.tile([D, 1], dt.float32)
            nc.sync.dma_start(out=w_enc_s[:], in_=Win_src[:])
            nc.sync.dma_start(out=w_enc_d[:], in_=Win_dst[:])
            nc.sync.dma_start(out=b_enc_s[:], in_=bin_src[:])
            nc.sync.dma_start(out=b_enc_d[:], in_=bin_dst[:])

            wbm_t, wu_t, bu_t = {}, {}, {}
            for l in range(L):
                for key, src in (("sd", Wbm_sd), ("ds", Wbm_ds)):
                    t = pers.tile([D + 1, D], dt.float16, name=f"wbm_{key}{l}")
                    nc.sync.dma_start(out=t[:], in_=src[l])
                    wbm_t[key, l] = t
                for key, src in (("dst", Wu_dst), ("src", Wu_src)):
                    th = pers.tile([D, D], dt.float16, name=f"wuh_{key}{l}")
                    ta = pers.tile([D, D], dt.float16, name=f"wua_{key}{l}")
                    nc.sync.dma_start(out=th[:], in_=src[l, 0:D])
                    nc.sync.dma_start(out=ta[:], in_=src[l, D:2 * D])
                    wu_t[key, l] = (th, ta)
                for key, src in (("dst", bu_dst), ("src", bu_src)):
                    t = pers.tile([D, 1], dt.float32, name=f"bu_{key}{l}")
                    nc.sync.dma_start(out=t[:], in_=src[l])
                    bu_t[key, l] = t

            for t in h_t.values():
                nc.vector.memset(t[D:D + 1, :], 1.0)

            # one-time zero fill of z-shard pad columns (never written later;
            # keeps AllGather/NaN checks clean)
            zeros64 = pers.tile([128, D], dt.float16, name="zeros64")
            nc.vector.memset(zeros64[:], 0.0)
            for z_shard, n in ((zs_shard, cfg.SRC_SH), (zd_shard, cfg.DST_SH)):
                for k in range(n // CHUNK):
                    nc.sync.dma_start(
                        out=z_shard[k * CHUNK:(k + 1) * CHUNK, D:128],
                        in_=zeros64[:])

            def encoder(xT, w_t, b_t, h_out, n):
                for j0 in range(0, n, WIN):
                    w = min(WIN, n - j0)
                    xs = work.tile([D, WIN], dt.float32, tag="xs")
                    nc.sync.dma_start(out=xs[:, :w], in_=xT[:, j0:j0 + w])
                    ps = ps_mlp.tile([D, WIN], dt.float32, tag="mlp")
                    nc.tensor.matmul(out=ps[:, :w], lhsT=w_t[:], rhs=xs[:, :w],
                                     start=True, stop=True)
                    nc.scalar.activation(out=h_out[0:D, j0:j0 + w],
                                         in_=ps[:, :w], func=RELU, bias=b_t[:])

            REPS = reps

            def z_phase(h_in, wbm, z_shard, n):
                for k in range(n // CHUNK):
                    ps = ps_mlp.tile([CHUNK, D], dt.float32, tag="mlp")
                    nc.tensor.matmul(
                        out=ps[:], lhsT=h_in[0:D + 1, k * CHUNK:(k + 1) * CHUNK],
                        rhs=wbm[:], start=True, stop=True)
                    zs = work.tile([CHUNK, D], dt.float16, tag="zstage")
                    nc.scalar.activation(out=zs[:], in_=ps[:], func=RELU)
                    nc.sync.dma_start(
                        out=z_shard[k * CHUNK:(k + 1) * CHUNK, 0:D], in_=zs[:])

            gather_count = [0]

            def sweep(plan, z_tabs, idx_dram, dw_t, agg_t, table_rows, shard):
                nc.vector.memset(agg_t[:], 0.0)
                gathers = plan["gathers"]
                gtiles = []
                for gno, (q, s0, nsl) in enumerate(gathers):
                    it = idxg.tile([128, nsl // 16], dt.int16, tag="idxg")
                    nc.sync.dma_start(
                        out=it[:], in_=idx_dram[:, s0 // 16:(s0 + nsl) // 16])
                    gt = gath.tile([128, (nsl // CHUNK) * 128], dt.float16,
                                   tag="gt")
                    nc.gpsimd.dma_gather(
                        gt[:].rearrange("p (b e) -> p b e", e=128),
                        z_tabs[q][:],
                        it[:], nsl, nsl, 128, single_packet=False,
                        queue_num=gather_count[0] % 4)
                    gather_count[0] += 1
                    gtiles.append((s0, nsl, gt))

                SB = 8  # chunks per S-build batch
                gi = 0
                chunk0 = 0  # global chunk cursor
                s_tile = None
                for (q, w, ks) in plan["visits"]:
                    nvis = sum(ks)
                    ps = ps_agg.tile([D, WIN], dt.float32, tag="agg")
                    nc.vector.memset(ps[:], 0.0)
                    done = 0
                    for si, kk in enumerate(ks):
                        for _ in range(kk):
                            c = chunk0
                            # S batch
                            if c % SB == 0:
                                nb = min(SB, dw_t.shape[1] - c)
                                s_tile = spool.tile([128, nb * SUB], dt.float16,
                                                    tag="s")
                                nc.vector.tensor_tensor(
                                    out=s_tile[:].rearrange(
                                        "p (b e) -> p b e", e=SUB),
                                    in0=dw_t[:, c:c + nb, None].to_broadcast(
                                        [128, nb, SUB]),
                                    in1=iota_t[:, None, :].to_broadcast(
                                        [128, nb, SUB]),
                                    op=EQ)
                            # gather tile & block for this chunk
                            s0, nsl, gt = gtiles[gi]
                            if c * CHUNK >= s0 + nsl:
                                gi += 1
                                s0, nsl, gt = gtiles[gi]
                            blk = (c * CHUNK - s0) // CHUNK
                            g3 = gt[:].rearrange("p (b e) -> p b e", e=128)
                            done += 1
                            nc.tensor.matmul(
                                out=ps[:, si * SUB:(si + 1) * SUB],
                                lhsT=g3[:, blk, 0:D],
                                rhs=s_tile[:].rearrange(
                                    "p (b e) -> p b e", e=SUB)[:, c % SB, :],
                                start=False, stop=(done == nvis),
                                skip_group_check=True)
                            chunk0 += 1
                    hi = min((w + 1) * WIN, shard)
                    nc.vector.tensor_add(
                        out=agg_t[:, w * WIN:hi], in0=agg_t[:, w * WIN:hi],
                        in1=ps[:, :hi - w * WIN])

            def update(h_io, agg_t, wu, bu, n):
                # split K: ps = Wu[0:D].T @ h + Wu[D:2D].T @ agg (no concat copy)
                for j0 in range(0, n, WIN):
                    w = min(WIN, n - j0)
                    ps = ps_mlp.tile([D, WIN], dt.float32, tag="mlp")
                    nc.tensor.matmul(out=ps[:, :w], lhsT=wu[0][:],
                                     rhs=h_io[0:D, j0:j0 + w],
                                     start=True, stop=False)
                    nc.tensor.matmul(out=ps[:, :w], lhsT=wu[1][:],
                                     rhs=agg_t[:, j0:j0 + w],
                                     start=False, stop=True)
                    nc.scalar.activation(out=h_io[0:D, j0:j0 + w],
                                         in_=ps[:, :w], func=RELU, bias=bu[:])

            for _rep in range(REPS):
              encoder(xT_src, w_enc_s, b_enc_s, h_s, cfg.SRC_SH)
              encoder(xT_dst, w_enc_d, b_enc_d, h_d, cfg.DST_SH)
              for l in range(L):
                z_phase(h_s, wbm_t["sd", l], zs_shard, cfg.SRC_SH)
                nc.gpsimd.collective_compute(
                    "AllGather", mybir.AluOpType.bypass, replica_groups=rg,
                    ins=[zs_shard.opt()], outs=[zs_full.opt()])
                for q in range(cfg.N_TAB_S):
                    nc.sync.dma_start(
                        out=zs_loc[q][:],
                        in_=zs_full[q * cfg.TAB_S:(q + 1) * cfg.TAB_S, :])
                sweep(plan_sd, zs_loc, idx_sd, dw_sd_t, agg_d,
                      cfg.TAB_S, cfg.DST_SH)
                update(h_d, agg_d, wu_t["dst", l], bu_t["dst", l], cfg.DST_SH)

                if l == L - 1:
                    nc.sync.dma_start(out=out_hd[:], in_=h_d[0:D, :])
                    break

                z_phase(h_d, wbm_t["ds", l], zd_shard, cfg.DST_SH)
                nc.gpsimd.collective_compute(
                    "AllGather", mybir.AluOpType.bypass, replica_groups=rg,
                    ins=[zd_shard.opt()], outs=[zd_full.opt()])
                for q in range(cfg.N_TAB_D):
                    nc.sync.dma_start(
                        out=zd_loc[q][:],
                        in_=zd_full[q * cfg.TAB_D:(q + 1) * cfg.TAB_D, :])
                sweep(plan_ds, zd_loc, idx_ds, dw_ds_t, agg_s,
                      cfg.TAB_D, cfg.SRC_SH)
                update(h_s, agg_s, wu_t["src", l], bu_t["src", l], cfg.SRC_SH)

    nc.compile()
    return nc


def make_in_maps(cfg, host):
    shared = dict(
        Win_src=host["Win_src"], Win_dst=host["Win_dst"],
        bin_src=host["bin_src"], bin_dst=host["bin_dst"],
        Wbm_sd=host["Wbm_sd"], Wbm_ds=host["Wbm_ds"],
        Wu_dst=host["Wu_dst"], Wu_src=host["Wu_src"],
        bu_dst=host["bu_dst"], bu_src=host["bu_src"],
        iota=host["iota"],
    )
    maps = []
    for c in range(N_CORES):
        m = dict(shared)
        m["xT_src"] = host["xsT"][c]
        m["xT_dst"] = host["xdT"][c]
        m["idx_sd"] = host["plan_sd"]["idx16"][c]
        m["dw_sd"] = host["plan_sd"]["dw"][c]
        m["idx_ds"] = host["plan_ds"]["idx16"][c]
        m["dw_ds"] = host["plan_ds"]["dw"][c]
        maps.append(m)
    return maps


LAST_RES = None


def kernel(**inputs) -> np.ndarray:
    global LAST_RES
    cfg = REAL_CFG
    host = _host_prep(cfg, inputs)
    nc = _build_nc(cfg, host)
    from concourse.bass_utils import run_bass_kernel_spmd
    res = run_bass_kernel_spmd(nc, make_in_maps(cfg, host),
                               core_ids=list(range(N_CORES)))
    LAST_RES = res
    nd = np.asarray(inputs["x_dst"]).shape[0]
    out = np.concatenate(
        [res.results[c]["out_hd"].T[host["perm_d"][c]] for c in range(N_CORES)],
        axis=0)[:nd]
    return out.astype(np.float32)



# revision 27
# speedup vs baseline: 1.6843x; 1.0013x over previous
"""Bipartite GNN message passing on 8 Trainium2 NeuronCores.

Math reformulation: relu(h[idx] @ W + b) == relu(h @ W + b)[idx], so each
direction-layer is: per-node message MLP (z) -> gather z rows by edge ->
segment-sum -> update MLP.  Sharding: aggregation-side nodes are split into
8 contiguous ranges (one per core); each core owns ALL edges targeting its
range, so it computes complete aggregates locally (no AllReduce).  Only the
small per-shard z tensors are AllGathered (fp16) each direction-layer.

Segment-sum on the tensor engine: edges sorted by (gather-table, subwindow);
for each 128-edge chunk a one-hot S[e,j] = (dst_local[e] == j) matrix (built
with one DVE is_equal against a constant iota tile) turns the segment sum
into  psum[64f, 64dst] += gathered_z[128e, 64f].T @ S[128e, 64dst],
accumulated per 512-dst PSUM window and flushed additively to SBUF.

Gathers use dma_gather (one SWDGE instruction per ~8k rows, int16 indices
into <=25088-row table slices of the AllGathered z, 256B padded fp16 rows).

SPMD: one NEFF for all 8 cores, so the chunk schedule (chunks per
(table, subwindow)) is the max over cores; cores pad with (idx=0,
dst_local=sentinel) edges that contribute zero.
"""
import numpy as np

D = 64
CHUNK = 128
SUB = 64
WIN = 512
SENT = 999.0
N_CORES = 8
GMAX = 1024


class Cfg:
    def __init__(self, ns_pad, nd_pad, n_tab_s, n_tab_d):
        self.NS_PAD, self.ND_PAD = ns_pad, nd_pad
        self.SRC_SH, self.DST_SH = ns_pad // N_CORES, nd_pad // N_CORES
        self.N_TAB_S, self.N_TAB_D = n_tab_s, n_tab_d
        self.TAB_S = ns_pad // n_tab_s
        self.TAB_D = nd_pad // n_tab_d
        assert self.TAB_S <= 32767 and self.TAB_D <= 32767
        assert self.SRC_SH % CHUNK == 0 and self.DST_SH % CHUNK == 0


REAL_CFG = Cfg(100352, 50176, 4, 2)


def _build_plan(gather_idx, seg_idx, table_rows, n_tables, shard):
    """SPMD-uniform edge plan for one direction.

    Returns dict with per-core idx16 [128, TOT/16] int16, dw [128, TOT/128]
    fp16, plus uniform visits [(q, w, [k_per_sub])] and gathers
    [(q, slot0, nslots)].
    """
    n_sub_tot = shard // SUB
    core_of = seg_idx // shard
    per_core = []
    Kmax = np.zeros((n_tables, n_sub_tot), np.int64)
    for c in range(N_CORES):
        m = core_of == c
        g = gather_idx[m]
        s = seg_idx[m] - c * shard
        q = (g // table_rows).astype(np.int64)
        subg = s // SUB
        key = q * n_sub_tot + subg
        order = np.argsort(key, kind="stable")
        g, s, q, subg = g[order], s[order], q[order], subg[order]
        per_core.append((g % table_rows, s, q, subg))
        cnt = np.zeros((n_tables, n_sub_tot), np.int64)
        np.add.at(cnt, (q, subg), 1)
        Kmax = np.maximum(Kmax, (cnt + CHUNK - 1) // CHUNK)

    slots_per_group = Kmax * CHUNK
    flat = slots_per_group.reshape(-1)
    starts = np.concatenate([[0], np.cumsum(flat)[:-1]]).reshape(n_tables, n_sub_tot)
    total = int(flat.sum())
    assert total % CHUNK == 0

    idx16_list, dw_list = [], []
    for c in range(N_CORES):
        g, s, q, subg = per_core[c]
        G = np.zeros(total, np.int32)
        DW = np.full(total, SENT, np.float32)
        kk = q * n_sub_tot + subg
        bounds = np.flatnonzero(np.diff(kk)) + 1
        for grp in np.split(np.arange(len(g)), bounds):
            if len(grp) == 0:
                continue
            qq, sg = int(q[grp[0]]), int(subg[grp[0]])
            st = int(starts[qq, sg])
            n = len(grp)
            G[st:st + n] = g[grp]
            DW[st:st + n] = s[grp] - sg * SUB
        # idx16: j -> [16k + j%16, j//16] replicated for 8 gpsimd cores
        i16 = np.empty((128, total // 16), np.int16)
        base = G.astype(np.int16).reshape(total // 16, 16).T  # [16, T/16]
        for k in range(8):
            i16[16 * k:16 * (k + 1)] = base
        dw = DW.astype(np.float16).reshape(total // CHUNK, CHUNK).T  # [128, T/128]
        idx16_list.append(i16)
        dw_list.append(np.ascontiguousarray(dw))

    n_win = (shard + WIN - 1) // WIN
    nsub_per_win = WIN // SUB
    visits = []
    for q in range(n_tables):
        for w in range(n_win):
            lo = w * nsub_per_win
            hi = min(lo + nsub_per_win, n_sub_tot)
            ks = [int(Kmax[q, sg]) for sg in range(lo, hi)]
            visits.append((q, w, ks))

    gathers = []
    for q in range(n_tables):
        q_lo = int(starts[q, 0])
        q_hi = int(starts[q, n_sub_tot - 1] + slots_per_group[q, n_sub_tot - 1])
        p = q_lo
        while p < q_hi:
            n = min(GMAX, q_hi - p)
            gathers.append((q, p, n))
            p += n
    return dict(idx16=idx16_list, dw=dw_list, visits=visits, gathers=gathers,
                total=total)


def _balance_relabel(gather_idx, seg_idx, table_rows, n_tables, shard):
    """Per-core node relabeling within each seg-side shard so per
    (gather-table, 64-sub) edge counts stay <= 4*CHUNK for all "hard" subs,
    with excess concentrated in the tail-window "overflow" subs (same subs
    on every core, so the SPMD Kmax stays 4 for hard groups).
    Returns perm[NCORES, shard]: perm[c, old] = new."""
    n_sub = shard // SUB
    n_over = n_sub - 8 * (n_sub // 8)  # tail-window subs
    if n_over == 0:
        n_over = 2
    n_hard = n_sub - n_over
    target = 4 * CHUNK
    core_of = seg_idx // shard
    q_of = gather_idx // table_rows
    perm = np.empty((N_CORES, shard), np.int64)
    for c in range(N_CORES):
        m = core_of == c
        loc = seg_idx[m] - c * shard
        qq = q_of[m]
        deg = np.zeros((shard, n_tables), np.int64)
        np.add.at(deg, (loc, qq), 1)
        order = np.argsort(-deg.sum(1), kind="stable")
        cnt = np.zeros((n_sub, n_tables), np.int64)
        cap = np.full(n_sub, SUB, np.int64)
        fill_pos = np.zeros(n_sub, np.int64)

        def place(node, b):
            perm[c, node] = b * SUB + fill_pos[b]
            fill_pos[b] += 1
            cnt[b] += deg[node]
            cap[b] -= 1

        # heaviest nodes -> overflow subs (they absorb the excess over the
        # 512 hard cap; overflow groups may have Kmax 5-8, that's fine)
        T = n_over * SUB
        for node in order[:T]:
            ov = np.arange(n_hard, n_sub)
            ovc = ov[cap[ov] > 0]
            b = int(ovc[np.argmin((cnt[ovc] + deg[node]).max(1))])
            place(node, b)
        # rest: best-fit into hard bins under the cap
        for node in order[T:]:
            d = deg[node]
            nc_ = cnt[:n_hard] + d
            feas = (nc_ <= target).all(1) & (cap[:n_hard] > 0)
            if feas.any():
                tot = nc_.sum(1)
                tot[~feas] = -1
                b = int(np.argmax(tot))
            else:
                sp = np.maximum(cnt + d - target, 0).sum(1)
                sp[cap == 0] = 1 << 40
                b = int(np.argmin(sp))
            place(node, b)
        _swap_repair(deg, perm[c], cnt, n_hard, target)
    return perm


def _swap_repair(deg, perm_c, cnt, n_hard, target, rounds=8):
    """Node-for-node swaps to push over-cap hard (bin, q) groups under
    `target`. Swapping perm values preserves bin capacities exactly."""
    bin_of = perm_c // SUB
    for _ in range(rounds):
        viol = np.argwhere(cnt[:n_hard] > target)
        if not len(viol):
            break
        fixed_any = False
        for b, q in viol:
            guard = 0
            while cnt[b, q] > target and guard < 16:
                guard += 1
                nodes_b = np.where(bin_of == b)[0]
                x = nodes_b[np.argmax(deg[nodes_b, q])]
                dx = deg[x]
                order_u = np.argsort(cnt[:n_hard, q])
                done = False
                for u in order_u[:24]:
                    if u == b:
                        continue
                    nodes_u = np.where(bin_of == u)[0]
                    need = np.maximum(cnt[u] + dx - target, 0)
                    ub = target - cnt[b] + dx
                    dnu = deg[nodes_u]
                    ok = ((dnu >= need).all(1) & (dnu <= ub).all(1)
                          & (dnu[:, q] < dx[q]))
                    cand = nodes_u[ok]
                    if len(cand):
                        y = cand[np.argmin(deg[cand].sum(1))]
                        bin_of[x], bin_of[y] = u, b
                        perm_c[x], perm_c[y] = perm_c[y], perm_c[x]
                        cnt[b] += deg[y] - dx
                        cnt[u] += dx - deg[y]
                        done = fixed_any = True
                        break
                if not done:
                    break
        if not fixed_any:
            break


def _host_prep(cfg, inputs):
    f32 = np.float32
    x_src = np.asarray(inputs["x_src"], f32)
    x_dst = np.asarray(inputs["x_dst"], f32)
    src_idx = np.asarray(inputs["src_idx"]).astype(np.int64)
    dst_idx = np.asarray(inputs["dst_idx"]).astype(np.int64)
    L = np.asarray(inputs["W_msg_sd"]).shape[0]

    ns, nd = x_src.shape[0], x_dst.shape[0]
    xs = np.zeros((cfg.NS_PAD, D), f32)
    xs[:ns] = x_src
    xd = np.zeros((cfg.ND_PAD, D), f32)
    xd[:nd] = x_dst

    # balance (table, sub) group counts across cores via node relabeling
    perm_d = _balance_relabel(src_idx, dst_idx, cfg.TAB_S, cfg.N_TAB_S,
                              cfg.DST_SH)
    perm_s = _balance_relabel(dst_idx, src_idx, cfg.TAB_D, cfg.N_TAB_D,
                              cfg.SRC_SH)
    dc = dst_idx // cfg.DST_SH
    dst_rel = dc * cfg.DST_SH + perm_d[dc, dst_idx % cfg.DST_SH]
    sc = src_idx // cfg.SRC_SH
    src_rel = sc * cfg.SRC_SH + perm_s[sc, src_idx % cfg.SRC_SH]
    # apply the same relabeling to node feature rows: new row k of shard c
    # holds old node argwhere(perm==k)
    inv_d = np.empty_like(perm_d)
    inv_s = np.empty_like(perm_s)
    for c in range(N_CORES):
        inv_d[c, perm_d[c]] = np.arange(cfg.DST_SH)
        inv_s[c, perm_s[c]] = np.arange(cfg.SRC_SH)
        xs[c * cfg.SRC_SH:(c + 1) * cfg.SRC_SH] = \
            xs[c * cfg.SRC_SH:(c + 1) * cfg.SRC_SH][inv_s[c]]
        xd[c * cfg.DST_SH:(c + 1) * cfg.DST_SH] = \
            xd[c * cfg.DST_SH:(c + 1) * cfg.DST_SH][inv_d[c]]

    plan_sd = _build_plan(src_rel, dst_rel, cfg.TAB_S, cfg.N_TAB_S, cfg.DST_SH)
    plan_ds = _build_plan(dst_rel, src_rel, cfg.TAB_D, cfg.N_TAB_D, cfg.SRC_SH)

    def stack_wb(wk, bk):
        w = np.asarray(inputs[wk], f32)
        b = np.asarray(inputs[bk], f32)
        out = np.empty((L, D + 1, D), np.float16)
        out[:, :D] = w.astype(np.float16)
        out[:, D] = b.astype(np.float16)
        return out

    host = dict(
        L=L,
        Wbm_sd=stack_wb("W_msg_sd", "b_msg_sd"),
        Wbm_ds=stack_wb("W_msg_ds", "b_msg_ds"),
        Wu_dst=np.asarray(inputs["W_upd_dst"], f32).astype(np.float16),
        Wu_src=np.asarray(inputs["W_upd_src"], f32).astype(np.float16),
        bu_dst=np.asarray(inputs["b_upd_dst"], f32)[:, :, None],
        bu_src=np.asarray(inputs["b_upd_src"], f32)[:, :, None],
        Win_src=np.asarray(inputs["W_in_src"], f32),
        Win_dst=np.asarray(inputs["W_in_dst"], f32),
        bin_src=np.asarray(inputs["b_in_src"], f32)[:, None],
        bin_dst=np.asarray(inputs["b_in_dst"], f32)[:, None],
        iota=np.tile(np.arange(SUB, dtype=np.float16), (128, 1)),
        xsT=[np.ascontiguousarray(xs[c * cfg.SRC_SH:(c + 1) * cfg.SRC_SH].T)
             for c in range(N_CORES)],
        xdT=[np.ascontiguousarray(xd[c * cfg.DST_SH:(c + 1) * cfg.DST_SH].T)
             for c in range(N_CORES)],
        plan_sd=plan_sd, plan_ds=plan_ds, perm_d=perm_d,
    )
    return host


def _build_nc(cfg, host, reps=1):
    import concourse.bass as bass
    import concourse.tile as tile
    from concourse import bacc, mybir

    dt = mybir.dt
    L = host["L"]
    plan_sd, plan_ds = host["plan_sd"], host["plan_ds"]
    TOT_SD, TOT_DS = plan_sd["total"], plan_ds["total"]

    nc = bacc.Bacc("TRN2", target_bir_lowering=False, debug=False,
                   num_devices=N_CORES, num_swdge_queues=4,
                   dynamic_dma_scratch_size=32768)

    def inp(name, shape, dtype):
        return nc.dram_tensor(name, shape, dtype, kind="ExternalInput").ap()

    xT_src = inp("xT_src", [D, cfg.SRC_SH], dt.float32)
    xT_dst = inp("xT_dst", [D, cfg.DST_SH], dt.float32)
    Win_src = inp("Win_src", [D, D], dt.float32)
    Win_dst = inp("Win_dst", [D, D], dt.float32)
    bin_src = inp("bin_src", [D, 1], dt.float32)
    bin_dst = inp("bin_dst", [D, 1], dt.float32)
    Wbm_sd = inp("Wbm_sd", [L, D + 1, D], dt.float16)
    Wbm_ds = inp("Wbm_ds", [L, D + 1, D], dt.float16)
    Wu_dst = inp("Wu_dst", [L, 2 * D, D], dt.float16)
    Wu_src = inp("Wu_src", [L, 2 * D, D], dt.float16)
    bu_dst = inp("bu_dst", [L, D, 1], dt.float32)
    bu_src = inp("bu_src", [L, D, 1], dt.float32)
    iota_in = inp("iota", [128, SUB], dt.float16)
    idx_sd = inp("idx_sd", [128, TOT_SD // 16], dt.int16)
    dw_sd = inp("dw_sd", [128, TOT_SD // CHUNK], dt.float16)
    idx_ds = inp("idx_ds", [128, TOT_DS // 16], dt.int16)
    dw_ds = inp("dw_ds", [128, TOT_DS // CHUNK], dt.float16)
    out_hd = nc.dram_tensor("out_hd", [D, cfg.DST_SH], dt.float16,
                            kind="ExternalOutput").ap()

    zs_shard = nc.dram_tensor("zs_shard", [cfg.SRC_SH, 128], dt.float16).ap()
    zd_shard = nc.dram_tensor("zd_shard", [cfg.DST_SH, 128], dt.float16).ap()
    zs_full = nc.dram_tensor("zs_full", [cfg.NS_PAD, 128], dt.float16,
                             addr_space="Shared").ap()
    zd_full = nc.dram_tensor("zd_full", [cfg.ND_PAD, 128], dt.float16,
                             addr_space="Shared").ap()
    # gathers from Shared-space DRAM are ~1.8x slower per descriptor than
    # local DRAM; copy each table locally after the AllGather (per-table
    # tensors so table-q gathers only wait on table-q's copy)
    zs_loc = [nc.dram_tensor(f"zs_loc{q}", [cfg.TAB_S, 128], dt.float16).ap()
              for q in range(cfg.N_TAB_S)]
    zd_loc = [nc.dram_tensor(f"zd_loc{q}", [cfg.TAB_D, 128], dt.float16).ap()
              for q in range(cfg.N_TAB_D)]

    RELU = mybir.ActivationFunctionType.Relu
    EQ = mybir.AluOpType.is_equal
    rg = [list(range(N_CORES))]

    with tile.TileContext(nc) as tc:
        from contextlib import ExitStack
        with ExitStack() as ctx:
            pers = ctx.enter_context(tc.tile_pool(name="pers", bufs=1))
            ps_agg = ctx.enter_context(
                tc.tile_pool(name="psagg", bufs=4, space="PSUM"))
            ps_mlp = ctx.enter_context(
                tc.tile_pool(name="psmlp", bufs=2, space="PSUM"))
            gath = ctx.enter_context(tc.tile_pool(name="gath", bufs=24))
            idxg = ctx.enter_context(tc.tile_pool(name="idxg", bufs=24))
            spool = ctx.enter_context(tc.tile_pool(name="spool", bufs=4))
            work = ctx.enter_context(tc.tile_pool(name="work", bufs=4))

            h_s = pers.tile([D + 1, cfg.SRC_SH], dt.float16, name="h_s")
            h_d = pers.tile([D + 1, cfg.DST_SH], dt.float16, name="h_d")
            h_t = {"s": h_s, "d": h_d}
            agg_s = pers.tile([D, cfg.SRC_SH], dt.float16)
            agg_d = pers.tile([D, cfg.DST_SH], dt.float16)
            iota_t = pers.tile([128, SUB], dt.float16)
            dw_sd_t = pers.tile([128, TOT_SD // CHUNK], dt.float16)
            dw_ds_t = pers.tile([128, TOT_DS // CHUNK], dt.float16)

            nc.sync.dma_start(out=iota_t[:], in_=iota_in[:])
            nc.sync.dma_start(out=dw_sd_t[:], in_=dw_sd[:])
            nc.sync.dma_start(out=dw_ds_t[:], in_=dw_ds[:])

            w_enc_s = pers.tile([D, D], dt.float32)
            w_enc_d = pers.tile([D, D], dt.float32)
            b_enc_s = pers.tile([D, 1], dt.float32)
            b_enc_d = pers.tile([D, 1], dt.float32)
            nc.sync.dma_start(out=w_enc_s[:], in_=Win_src[:])
            nc.sync.dma_start(out=w_enc_d[:], in_=Win_dst[:])
            nc.sync.dma_start(out=b_enc_s[:], in_=bin_src[:])
            nc.sync.dma_start(out=b_enc_d[:], in_=bin_dst[:])

            wbm_t, wu_t, bu_t = {}, {}, {}
            for l in range(L):
                for key, src in (("sd", Wbm_sd), ("ds", Wbm_ds)):
                    t = pers.tile([D + 1, D], dt.float16, name=f"wbm_{key}{l}")
                    nc.sync.dma_start(out=t[:], in_=src[l])
                    wbm_t[key, l] = t
                for key, src in (("dst", Wu_dst), ("src", Wu_src)):
                    th = pers.tile([D, D], dt.float16, name=f"wuh_{key}{l}")
                    ta = pers.tile([D, D], dt.float16, name=f"wua_{key}{l}")
                    nc.sync.dma_start(out=th[:], in_=src[l, 0:D])
                    nc.sync.dma_start(out=ta[:], in_=src[l, D:2 * D])
                    wu_t[key, l] = (th, ta)
                for key, src in (("dst", bu_dst), ("src", bu_src)):
                    t = pers.tile([D, 1], dt.float32, name=f"bu_{key}{l}")
                    nc.sync.dma_start(out=t[:], in_=src[l])
                    bu_t[key, l] = t

            for t in h_t.values():
                nc.vector.memset(t[D:D + 1, :], 1.0)

            # one-time zero fill of z-shard pad columns (never written later;
            # keeps AllGather/NaN checks clean)
            zeros64 = pers.tile([128, D], dt.float16, name="zeros64")
            nc.vector.memset(zeros64[:], 0.0)
            for z_shard, n in ((zs_shard, cfg.SRC_SH), (zd_shard, cfg.DST_SH)):
                for k in range(n // CHUNK):
                    nc.sync.dma_start(
                        out=z_shard[k * CHUNK:(k + 1) * CHUNK, D:128],
                        in_=zeros64[:])

            def encoder(xT, w_t, b_t, h_out, n):
                for j0 in range(0, n, WIN):
                    w = min(WIN, n - j0)
                    xs = work.tile([D, WIN], dt.float32, tag="xs")
                    nc.sync.dma_start(out=xs[:, :w], in_=xT[:, j0:j0 + w])
                    ps = ps_mlp.tile([D, WIN], dt.float32, tag="mlp")
                    nc.tensor.matmul(out=ps[:, :w], lhsT=w_t[:], rhs=xs[:, :w],
                                     start=True, stop=True)
                    nc.scalar.activation(out=h_out[0:D, j0:j0 + w],
                                         in_=ps[:, :w], func=RELU, bias=b_t[:])

            REPS = reps

            def z_phase(h_in, wbm, z_shard, n):
                for k in range(n // CHUNK):
                    ps = ps_mlp.tile([CHUNK, D], dt.float32, tag="mlp")
                    nc.tensor.matmul(
                        out=ps[:], lhsT=h_in[0:D + 1, k * CHUNK:(k + 1) * CHUNK],
                        rhs=wbm[:], start=True, stop=True)
                    zs = work.tile([CHUNK, D], dt.float16, tag="zstage")
                    nc.scalar.activation(out=zs[:], in_=ps[:], func=RELU)
                    nc.sync.dma_start(
                        out=z_shard[k * CHUNK:(k + 1) * CHUNK, 0:D], in_=zs[:])

            gather_count = [0]

            def sweep(plan, z_tabs, idx_dram, dw_t, agg_t, table_rows, shard):
                nc.vector.memset(agg_t[:], 0.0)
                gathers = plan["gathers"]
                gtiles = []
                for gno, (q, s0, nsl) in enumerate(gathers):
                    it = idxg.tile([128, nsl // 16], dt.int16, tag="idxg")
                    nc.sync.dma_start(
                        out=it[:], in_=idx_dram[:, s0 // 16:(s0 + nsl) // 16])
                    gt = gath.tile([128, (nsl // CHUNK) * 128], dt.float16,
                                   tag="gt")
                    nc.gpsimd.dma_gather(
                        gt[:].rearrange("p (b e) -> p b e", e=128),
                        z_tabs[q][:],
                        it[:], nsl, nsl, 128, single_packet=False,
                        queue_num=gather_count[0] % 4)
                    gather_count[0] += 1
                    gtiles.append((s0, nsl, gt))

                SB = 8  # chunks per S-build batch
                gi = 0
                chunk0 = 0  # global chunk cursor
                s_tile = None
                for (q, w, ks) in plan["visits"]:
                    nvis = sum(ks)
                    ps = ps_agg.tile([D, WIN], dt.float32, tag="agg")
                    nc.vector.memset(ps[:], 0.0)
                    done = 0
                    for si, kk in enumerate(ks):
                        for _ in range(kk):
                            c = chunk0
                            # S batch
                            if c % SB == 0:
                                nb = min(SB, dw_t.shape[1] - c)
                                s_tile = spool.tile([128, nb * SUB], dt.float16,
                                                    tag="s")
                                nc.vector.tensor_tensor(
                                    out=s_tile[:].rearrange(
                                        "p (b e) -> p b e", e=SUB),
                                    in0=dw_t[:, c:c + nb, None].to_broadcast(
                                        [128, nb, SUB]),
                                    in1=iota_t[:, None, :].to_broadcast(
                                        [128, nb, SUB]),
                                    op=EQ)
                            # gather tile & block for this chunk
                            s0, nsl, gt = gtiles[gi]
                            if c * CHUNK >= s0 + nsl:
                                gi += 1
                                s0, nsl, gt = gtiles[gi]
                            blk = (c * CHUNK - s0) // CHUNK
                            g3 = gt[:].rearrange("p (b e) -> p b e", e=128)
                            done += 1
                            nc.tensor.matmul(
                                out=ps[:, si * SUB:(si + 1) * SUB],
                                lhsT=g3[:, blk, 0:D],
                                rhs=s_tile[:].rearrange(
                                    "p (b e) -> p b e", e=SUB)[:, c % SB, :],
                                start=False, stop=(done == nvis),
                                skip_group_check=True)
                            chunk0 += 1
                    hi = min((w + 1) * WIN, shard)
                    nc.vector.tensor_add(
                        out=agg_t[:, w * WIN:hi], in0=agg_t[:, w * WIN:hi],
                        in1=ps[:, :hi - w * WIN])

            def update(h_io, agg_t, wu, bu, n):
                # split K: ps = Wu[0:D].T @ h + Wu[D:2D].T @ agg (no concat copy)
                for j0 in range(0, n, WIN):
                    w = min(WIN, n - j0)
                    ps = ps_mlp.tile([D, WIN], dt.float32, tag="mlp")
                    nc.tensor.matmul(out=ps[:, :w], lhsT=wu[0][:],
                                     rhs=h_io[0:D, j0:j0 + w],
                                     start=True, stop=False)
                    nc.tensor.matmul(out=ps[:, :w], lhsT=wu[1][:],
                                     rhs=agg_t[:, j0:j0 + w],
                                     start=False, stop=True)
                    nc.scalar.activation(out=h_io[0:D, j0:j0 + w],
                                         in_=ps[:, :w], func=RELU, bias=bu[:])

            for _rep in range(REPS):
              encoder(xT_src, w_enc_s, b_enc_s, h_s, cfg.SRC_SH)
              encoder(xT_dst, w_enc_d, b_enc_d, h_d, cfg.DST_SH)
              for l in range(L):
                z_phase(h_s, wbm_t["sd", l], zs_shard, cfg.SRC_SH)
                nc.gpsimd.collective_compute(
                    "AllGather", mybir.AluOpType.bypass, replica_groups=rg,
                    ins=[zs_shard.opt()], outs=[zs_full.opt()])
                for q in range(cfg.N_TAB_S):
                    nc.sync.dma_start(
                        out=zs_loc[q][:],
                        in_=zs_full[q * cfg.TAB_S:(q + 1) * cfg.TAB_S, :])
                sweep(plan_sd, zs_loc, idx_sd, dw_sd_t, agg_d,
                      cfg.TAB_S, cfg.DST_SH)
                update(h_d, agg_d, wu_t["dst", l], bu_t["dst", l], cfg.DST_SH)

                if l == L - 1:
                    nc.sync.dma_start(out=out_hd[:], in_=h_d[0:D, :])
                    break

                z_phase(h_d, wbm_t["ds", l], zd_shard, cfg.DST_SH)
                nc.gpsimd.collective_compute(
                    "AllGather", mybir.AluOpType.bypass, replica_groups=rg,
                    ins=[zd_shard.opt()], outs=[zd_full.opt()])
                for q in range(cfg.N_TAB_D):
                    nc.sync.dma_start(
                        out=zd_loc[q][:],
                        in_=zd_full[q * cfg.TAB_D:(q + 1) * cfg.TAB_D, :])
                sweep(plan_ds, zd_loc, idx_ds, dw_ds_t, agg_s,
                      cfg.TAB_D, cfg.SRC_SH)
                update(h_s, agg_s, wu_t["src", l], bu_t["src", l], cfg.SRC_SH)

    nc.compile()
    return nc


def make_in_maps(cfg, host):
    shared = dict(
        Win_src=host["Win_src"], Win_dst=host["Win_dst"],
        bin_src=host["bin_src"], bin_dst=host["bin_dst"],
        Wbm_sd=host["Wbm_sd"], Wbm_ds=host["Wbm_ds"],
        Wu_dst=host["Wu_dst"], Wu_src=host["Wu_src"],
        bu_dst=host["bu_dst"], bu_src=host["bu_src"],
        iota=host["iota"],
    )
    maps = []
    for c in range(N_CORES):
        m = dict(shared)
        m["xT_src"] = host["xsT"][c]
        m["xT_dst"] = host["xdT"][c]
        m["idx_sd"] = host["plan_sd"]["idx16"][c]
        m["dw_sd"] = host["plan_sd"]["dw"][c]
        m["idx_ds"] = host["plan_ds"]["idx16"][c]
        m["dw_ds"] = host["plan_ds"]["dw"][c]
        maps.append(m)
    return maps


LAST_RES = None


def kernel(**inputs) -> np.ndarray:
    global LAST_RES
    cfg = REAL_CFG
    host = _host_prep(cfg, inputs)
    nc = _build_nc(cfg, host)
    from concourse.bass_utils import run_bass_kernel_spmd
    res = run_bass_kernel_spmd(nc, make_in_maps(cfg, host),
                               core_ids=list(range(N_CORES)))
    LAST_RES = res
    nd = np.asarray(inputs["x_dst"]).shape[0]
    out = np.concatenate(
        [res.results[c]["out_hd"].T[host["perm_d"][c]] for c in range(N_CORES)],
        axis=0)[:nd]
    return out.astype(np.float32)

